# revision 65
# baseline (speedup 1.0000x reference)
"""AdaptiveTokenSampling Trainium2 kernel.

Data-parallel over batch: each of the 8 NeuronCores processes one batch
element end-to-end (per-row gumbel argmax sampling, sort-free dedup via
presence/rank matmuls, dma_gather of the selected attn rows).

Problem shapes (hardcoded): b=8, h=12, n=1024, d=64, k=256.
"""

import sys

for _p in ("/opt/trn_rl_repo", "/root/.axon_site/_ro/trn_rl_repo"):
    if _p not in sys.path:
        sys.path.append(_p)

import numpy as np

B, H, N, D, K = 8, 12, 1024, 64, 256
KP1 = K + 1                     # 257 output tokens (cls + k)
NROWS = H * N                   # 12288 flattened attn rows per core
GROWS = H * KP1                 # 3084 gathered rows per core
GPAD = 3200                     # 25 * 128
EPS = 1e-6
NEG_BIG = -1.0e30
MASK_VAL = -float(np.finfo(np.float32).max) / 2
POLY_THRESH = 1.0 / 32.0


def build_nc(debug=False):
    """Build the single-core Bass graph (same program for all 8 cores)."""
    import concourse.bacc as bacc
    import concourse.bass as bass
    import concourse.mybir as mybir
    from concourse.tile import TileContext
    from concourse.masks import make_identity

    f32 = mybir.dt.float32
    bf16 = mybir.dt.bfloat16
    i32 = mybir.dt.int32
    i16 = mybir.dt.int16
    u8 = mybir.dt.uint8
    Alu = mybir.AluOpType
    Act = mybir.ActivationFunctionType
    X = mybir.AxisListType.X

    nc = bacc.Bacc(num_swdge_queues=4)

    attn = nc.declare_dram_parameter("attn", [NROWS, N], f32, isOutput=False)
    # value is host-permuted to [128, 96*64]: partition-major so each
    # partition's 24KB loads as one contiguous DMA run
    value = nc.declare_dram_parameter("value", [128, 96 * D], f32, isOutput=False)
    gum = nc.declare_dram_parameter("gumbel", [K, N - 1], f32, isOutput=False)
    # mask is host-packed to column layout [128, 8]: mcol[p, j] = mask[128 j + p]
    maskp = nc.declare_dram_parameter("mask", [128, 8], f32, isOutput=False)
    maskp_v = maskp[:, :]
    out_attn = nc.declare_dram_parameter("out_attn", [GROWS, N], f32, isOutput=True)
    out_ids = nc.declare_dram_parameter("out_ids", [KP1], i32, isOutput=True)
    out_mask = nc.declare_dram_parameter("out_mask", [KP1], u8, isOutput=True)
    idr_scratch = nc.dram_tensor("idr_scratch", [GPAD], i16)
    if debug:
        dbg_cnt = nc.declare_dram_parameter("dbg_cnt", [128, 8], f32, isOutput=True)
        dbg_pos = nc.declare_dram_parameter("dbg_pos", [128, 8], f32, isOutput=True)
        dbg_pref = nc.declare_dram_parameter("dbg_pref", [128, 8], f32, isOutput=True)
        dbg_sv = nc.declare_dram_parameter("dbg_sv", [8], f32, isOutput=True)
        dbg_logits = nc.declare_dram_parameter("dbg_logits", [N], f32, isOutput=True)
        dbg_score = nc.declare_dram_parameter("dbg_score", [2, 128, N], f32,
                                              isOutput=True)

    with TileContext(nc) as tc:
        with (
            tc.tile_pool(name="const", bufs=1) as cpool,
            tc.tile_pool(name="vload", bufs=2) as vpool,
            tc.tile_pool(name="work", bufs=1) as wpool,
            tc.tile_pool(name="jt", bufs=2) as jpool,
            tc.tile_pool(name="gather", bufs=3) as gpool,
            tc.tile_pool(name="psA", bufs=1, space="PSUM") as ppool,
            tc.tile_pool(name="psB", bufs=2, space="PSUM") as spool,
        ):
            # ---- constants ----
            identity = cpool.tile([128, 128], f32, tag="identity")
            make_identity(nc, identity[:])
            ones_col = cpool.tile([128, 1], f32, tag="ones_col")
            nc.vector.memset(ones_col[:], 1.0)
            ones_col_bf = cpool.tile([128, 1], bf16, tag="ones_col_bf")
            nc.vector.memset(ones_col_bf[:], 1.0)
            ones_row = cpool.tile([1, 128], f32, tag="ones_row")
            nc.vector.memset(ones_row[:], 1.0)
            eps_col = cpool.tile([128, 1], f32, tag="eps_col")
            nc.vector.memset(eps_col[:], EPS)
            iota_i = cpool.tile([128, N], i32, tag="iota_i")
            nc.gpsimd.iota(iota_i[:], [[1, N]], channel_multiplier=0)
            iota_f = cpool.tile([128, N], f32, tag="iota_f")
            nc.vector.tensor_copy(iota_f[:], iota_i[:])
            offs_i = cpool.tile([12, 1], i32, tag="offs_i")
            nc.gpsimd.iota(offs_i[:], [[0, 1]], channel_multiplier=N)
            offs_f = cpool.tile([12, 1], f32, tag="offs_f")
            nc.vector.tensor_copy(offs_f[:], offs_i[:])
            # negsl[k, n] = -(k >= n), 8x8, for the exclusive-scan matmul
            negsl = cpool.tile([8, 8], f32, tag="negsl")
            nc.gpsimd.memset(negsl[:], -1.0)
            nc.gpsimd.affine_select(
                out=negsl[:], in_=negsl[:], compare_op=Alu.is_ge, fill=0.0,
                base=0, pattern=[[-1, 8]], channel_multiplier=1,
            )
            iota8_i = cpool.tile([8, 128], i32, tag="iota8_i")
            nc.gpsimd.iota(iota8_i[:], [[1, 128]], channel_multiplier=128)
            iota8f = cpool.tile([8, 128], f32, tag="iota8f")
            nc.vector.tensor_copy(iota8f[:], iota8_i[:])

            # ---- gumbel tiles: t2 = Ln(-(log1p-accurate Ln(u+eps)) + eps) ----
            # (independent of logits; runs while value norms load/compute)
            t2_tiles = []
            for jt in range(2):
                gt = jpool.tile([128, N], f32, tag="gt")
                nc.vector.memset(gt[:, 0:1], 0.0)
                geng = nc.scalar if jt == 0 else nc.sync
                geng.dma_start(out=gt[:, 1:N], in_=gum[jt * 128:(jt + 1) * 128, :])
                nc.vector.tensor_scalar(out=gt[:], in0=gt[:], scalar1=EPS,
                                        scalar2=None, op0=Alu.add)  # v = u + eps
                x = jpool.tile([128, N], f32, tag="x")
                nc.scalar.activation(x[:], gt[:], Act.Copy, bias=-1.0)
                t1a = jpool.tile([128, N], f32, tag="t1a")
                nc.scalar.activation(t1a[:], gt[:], Act.Ln)
                # cubic log1p for |x| < 1/32: x*(1 + x*(-1/2 + x/3))
                h1 = jpool.tile([128, N], f32, tag="h1")
                nc.vector.tensor_scalar(out=h1[:], in0=x[:], scalar1=1.0 / 3.0,
                                        scalar2=-0.5, op0=Alu.mult, op1=Alu.add)
                nc.vector.tensor_tensor(out=h1[:], in0=h1[:], in1=x[:], op=Alu.mult)
                nc.scalar.activation(h1[:], h1[:], Act.Copy, bias=1.0)
                nc.vector.tensor_tensor(out=h1[:], in0=h1[:], in1=x[:], op=Alu.mult)
                nc.scalar.activation(x[:], x[:], Act.Abs)
                selm = jpool.tile([128, N], u8, tag="selm")
                nc.vector.tensor_scalar(out=selm[:], in0=x[:], scalar1=POLY_THRESH,
                                        scalar2=None, op0=Alu.is_lt)
                nc.vector.copy_predicated(out=t1a[:], mask=selm[:], data=h1[:])
                nc.scalar.activation(t1a[:], t1a[:], Act.Ln,
                                     bias=eps_col[:, 0:1], scale=-1.0)  # t2
                t2_tiles.append(t1a)

            # ---- value norms: vnorm[p, c] = ||value row 128c+p||^2, c = 8h+j ----
            vview = value[:, :].rearrange("p (c d) -> p c d", d=D)  # [128,96,64]
            vnorm = wpool.tile([128, 96], f32, tag="vnorm")
            CH = 24
            for cc in range(96 // CH):
                vt = vpool.tile([128, CH, D], f32, tag="vt")
                eng = nc.sync if cc % 2 == 0 else nc.scalar
                eng.dma_start(out=vt[:, :, :], in_=vview[:, cc * CH:(cc + 1) * CH, :])
                sq = vpool.tile([128, CH, D], f32, tag="sq")
                nc.scalar.activation(sq[:, :, :], vt[:, :, :], Act.Square)
                nc.vector.tensor_reduce(
                    out=vnorm[:, cc * CH:(cc + 1) * CH], in_=sq[:, :, :],
                    axis=X, op=Alu.add,
                )
            nc.scalar.activation(vnorm[:], vnorm[:], Act.Sqrt)

            # ---- cls attn row ----
            a0n = wpool.tile([12, N], f32, tag="a0n")
            a0src = attn[:, :].rearrange("(h t) n -> h t n", t=N)[:, 0, :]  # [12, N]
            nc.sync.dma_start(out=a0n[:, :], in_=a0src)
            nc.vector.memset(a0n[:, 0:1], 0.0)  # exclude t=0
            a0T = wpool.tile([128, 96], f32, tag="a0T")
            a0T_v = a0T[:].rearrange("p (h j) -> p h j", h=12)
            for j8 in range(8):
                tp = spool.tile([128, 12], f32, tag="ps_scratch")
                nc.tensor.transpose(out=tp[:], in_=a0n[:, j8 * 128:(j8 + 1) * 128],
                                    identity=identity[:12, :12])
                nc.vector.tensor_copy(a0T_v[:, :, j8], tp[:])

            # ---- cls[p, j] = sum_h a0T * vnorm ----
            nc.vector.tensor_tensor(out=a0T[:], in0=a0T[:], in1=vnorm[:], op=Alu.mult)
            cls = wpool.tile([128, 8], f32, tag="cls")
            nc.vector.tensor_reduce(
                out=cls[:], in_=a0T[:].rearrange("p (h j) -> p j h", h=12),
                axis=X, op=Alu.add,
            )

            # ---- logits in column layout ----
            csum = wpool.tile([128, 1], f32, tag="csum")
            nc.vector.tensor_reduce(out=csum[:], in_=cls[:], axis=X, op=Alu.add)
            tot_ps = spool.tile([1, 1], f32, tag="ps_small")
            nc.tensor.matmul(tot_ps[:], lhsT=csum[:], rhs=ones_col[:, 0:1],
                             start=True, stop=True)
            tot_sb = wpool.tile([1, 1], f32, tag="tot_sb")
            nc.vector.tensor_copy(tot_sb[:], tot_ps[:])
            nc.vector.tensor_scalar(out=tot_sb[:], in0=tot_sb[:], scalar1=EPS,
                                    scalar2=None, op0=Alu.add)
            nc.vector.reciprocal(tot_sb[:], tot_sb[:])
            totb = wpool.tile([128, 1], f32, tag="totb")
            nc.gpsimd.partition_broadcast(totb[:], tot_sb[:])

            lm = wpool.tile([128, 8], f32, tag="lm")
            nc.vector.tensor_scalar(out=lm[:], in0=cls[:], scalar1=totb[:, 0:1],
                                    scalar2=None, op0=Alu.mult)
            nc.scalar.activation(lm[:], lm[:], Act.Ln, bias=eps_col[:, 0:1], scale=1.0)
            mcol = wpool.tile([128, 8], f32, tag="mcol")
            nc.sync.dma_start(out=mcol[:, :], in_=maskp_v)
            im = wpool.tile([128, 8], f32, tag="im")
            nc.vector.tensor_scalar(out=im[:], in0=mcol[:], scalar1=-MASK_VAL,
                                    scalar2=MASK_VAL, op0=Alu.mult, op1=Alu.add)
            nc.vector.tensor_tensor(out=lm[:], in0=lm[:], in1=mcol[:], op=Alu.mult)
            nc.vector.tensor_tensor(out=lm[:], in0=lm[:], in1=im[:], op=Alu.add)
            nc.vector.memset(lm[0:1, 0:1], NEG_BIG)  # t=0 excluded

            # ---- logits -> row -> broadcast over partitions via PE ----
            lrow8 = spool.tile([8, 128], f32, tag="ps_scratch")
            nc.tensor.transpose(out=lrow8[:], in_=lm[:], identity=identity[:])
            lrow8_sb = wpool.tile([8, 128], f32, tag="lrow8_sb")
            nc.vector.tensor_copy(lrow8_sb[:], lrow8[:])
            lrow = wpool.tile([1, N], f32, tag="lrow")
            nc.sync.dma_start(
                out=lrow[0:1, :].rearrange("x (j p) -> x j p", p=128),
                in_=lrow8_sb[:, :],
            )
            lb_ps = ppool.tile([128, N], f32, tag="ps_lb")
            for half in range(2):
                sl = slice(half * 512, (half + 1) * 512)
                nc.tensor.matmul(lb_ps[:, sl], lhsT=ones_row[0:1, :],
                                 rhs=lrow[0:1, sl], start=True, stop=True)

            # ---- scores + per-slot counts (cnt8[j, p] = count for t=128j+p) ----
            cnt8_tiles = []
            for jt in range(2):
                score = jpool.tile([128, N], f32, tag="gt")  # reuse slot rotation
                nc.vector.tensor_tensor(out=score[:], in0=lb_ps[:],
                                        in1=t2_tiles[jt][:], op=Alu.subtract)
                maxv = jpool.tile([128, 1], f32, tag="maxv")
                nc.vector.tensor_reduce(out=maxv[:], in_=score[:], axis=X, op=Alu.max)
                oh = jpool.tile([128, N], bf16, tag="ohbf")
                nc.vector.tensor_scalar(out=oh[:], in0=score[:],
                                        scalar1=maxv[:, 0:1], scalar2=None,
                                        op0=Alu.is_equal)
                if debug:
                    nc.sync.dma_start(out=dbg_score[jt, :, :], in_=score[:, :])
                cnt8_jt = ppool.tile([128, 8], f32, tag=f"ps_cnt8{jt}")
                cnt8_tiles.append(cnt8_jt)
                for j in range(8):
                    nc.tensor.matmul(cnt8_jt[:, j:j + 1],
                                     lhsT=oh[:, j * 128:(j + 1) * 128],
                                     rhs=ones_col_bf[:, 0:1],
                                     start=True, stop=True)

            # ---- presence / rank / position ([128,8] col -> [8,128] scan) ----
            cnt8a = wpool.tile([128, 8], f32, tag="cnt8a")
            nc.vector.tensor_copy(cnt8a[:], cnt8_tiles[0][:])
            nc.vector.tensor_tensor(out=cnt8a[:], in0=cnt8a[:],
                                    in1=cnt8_tiles[1][:], op=Alu.add)
            prescol = wpool.tile([128, 8], f32, tag="prescol")
            nc.vector.tensor_scalar(out=prescol[:], in0=cnt8a[:], scalar1=0.5,
                                    scalar2=None, op0=Alu.is_ge)
            pres8_ps = spool.tile([8, 128], f32, tag="ps_scratch")
            nc.tensor.transpose(out=pres8_ps[:], in_=prescol[:, :],
                                identity=identity[:])
            pres8 = pres8_ps
            scan8 = wpool.tile([8, 128], f32, tag="scan8")
            nc.vector.tensor_tensor_scan(
                out=scan8[:], data0=pres8[:], data1=iota8f[:, :],
                initial=0.0, op0=Alu.add, op1=Alu.bypass,
            )
            # svT[j] = excl[j] - m  via negsl[k, j] = -(k >= j)
            svT_ps = spool.tile([8, 1], f32, tag="ps_small")
            nc.tensor.matmul(svT_ps[:], lhsT=negsl[:8, :8], rhs=scan8[:, 127:128],
                             start=True, stop=True)
            svT = wpool.tile([8, 1], f32, tag="svT")
            nc.vector.tensor_copy(svT[:], svT_ps[:])
            # pos[j, p] = scan8 + (excl - m) + 256 ; mt = t * pres
            pos8 = wpool.tile([8, 128], f32, tag="pos8")
            nc.vector.tensor_scalar(out=pos8[:], in0=scan8[:], scalar1=svT[:, 0:1],
                                    scalar2=float(K), op0=Alu.add, op1=Alu.add)
            mt8 = wpool.tile([8, 128], f32, tag="mt8")
            nc.vector.tensor_tensor(out=mt8[:], in0=iota8f[:, :], in1=pres8[:],
                                    op=Alu.mult)
            pm_ps = spool.tile([128, 16], f32, tag="ps_scratch")
            nc.tensor.transpose(out=pm_ps[:, 0:8], in_=pos8[:, :],
                                identity=identity[:8, :8])
            nc.tensor.transpose(out=pm_ps[:, 8:16], in_=mt8[:, :],
                                identity=identity[:8, :8])
            pm = wpool.tile([128, 16], f32, tag="pm")
            nc.vector.tensor_copy(pm[:], pm_ps[:])
            if debug:
                nc.sync.dma_start(out=dbg_cnt[:, :], in_=pm[:, 0:8])
                nc.sync.dma_start(out=dbg_pos[:, :], in_=pm[:, 0:8])
                nc.sync.dma_start(out=dbg_pref[:, :], in_=pm[:, 8:16])
                nc.sync.dma_start(out=dbg_sv[:], in_=svT[0:1, 0:1])
                nc.sync.dma_start(out=dbg_logits[:], in_=lrow[0:1, :])

            # ---- scatter ids: ids[pos[p,j]] = t(p,j) for present entries ----
            ohp_all = wpool.tile([128, 8, KP1], f32, tag="ohp_all")
            nc.vector.tensor_tensor(
                out=ohp_all[:, :, :],
                in0=iota_f[:, :KP1].rearrange("p (o n) -> p o n", o=1).to_broadcast(
                    [128, 8, KP1]),
                in1=pm[:, 0:8].rearrange("p (j o) -> p j o", o=1).to_broadcast(
                    [128, 8, KP1]),
                op=Alu.is_equal,
            )
            ids_ps = spool.tile([1, KP1], f32, tag="ps_small")
            for j in range(8):
                nc.tensor.matmul(ids_ps[:], lhsT=pm[:, 8 + j:9 + j],
                                 rhs=ohp_all[:, j, :],
                                 start=(j == 0), stop=(j == 7))
            ids_f = wpool.tile([1, KP1], f32, tag="ids_f")
            nc.vector.tensor_copy(ids_f[:], ids_ps[:])
            maskf = wpool.tile([1, KP1], f32, tag="maskf")
            nc.vector.tensor_scalar(out=maskf[:], in0=ids_f[:], scalar1=0.5,
                                    scalar2=None, op0=Alu.is_ge)
            nc.vector.memset(maskf[0:1, 0:1], 1.0)
            ids_i = wpool.tile([1, KP1], i32, tag="ids_i")
            nc.vector.tensor_copy(ids_i[:], ids_f[:])
            mask_u = wpool.tile([1, KP1], u8, tag="mask_u")
            nc.vector.tensor_copy(mask_u[:], maskf[:])
            nc.sync.dma_start(out=out_ids[:], in_=ids_i[0:1, :])
            nc.sync.dma_start(out=out_mask[:], in_=mask_u[0:1, :])

            # ---- gather index list: flat = 257h + q -> row 1024h + ids[q] ----
            ids12 = wpool.tile([12, KP1], f32, tag="ids12")
            nc.gpsimd.partition_broadcast(ids12[:], ids_f[0:1, :])
            idr16 = wpool.tile([12, KP1], i16, tag="idr16")
            nc.vector.tensor_scalar(out=idr16[:], in0=ids12[:],
                                    scalar1=offs_f[:, 0:1], scalar2=None, op0=Alu.add)
            nc.sync.dma_start(out=idr_scratch[0:GROWS], in_=idr16[:, :])
            padt = wpool.tile([1, GPAD - GROWS], i16, tag="padt")
            nc.vector.memset(padt[:], -1)
            nc.scalar.dma_start(out=idr_scratch[GROWS:GPAD], in_=padt[0:1, :])
            # wrapped idxs: idxw[p, s] = idr[16 s + p]. dma_gather on queue q
            # reads only partitions [32q+16, 32q+32); also fill [0:16] for the
            # simulator's model.
            idxw = wpool.tile([128, GPAD // 16], i16, tag="idxw")
            nc.gpsimd.memset(idxw[:], 0)
            wrapped_src = idr_scratch[:].rearrange("(s p) -> p s", p=16)

            # ---- gather + store, chunks of 512 rows (+ final 12), queue 0 ----
            CHUNK = 512
            SC = CHUNK // 16
            for c in range(6):
                eng = nc.sync if c % 2 == 0 else nc.scalar
                eng.dma_start(out=idxw[16:32, c * SC:(c + 1) * SC],
                              in_=wrapped_src[:, c * SC:(c + 1) * SC])
                eng.dma_start(out=idxw[0:16, c * SC:(c + 1) * SC],
                              in_=wrapped_src[:, c * SC:(c + 1) * SC])
                gt = gpool.tile([128, CHUNK // 128, N], f32, tag="gchunk")
                nc.gpsimd.dma_gather(
                    out_ap=gt[:, :, :], in_ap=attn[:, :],
                    idxs_ap=idxw[:, c * SC:(c + 1) * SC],
                    num_idxs=CHUNK, num_idxs_reg=CHUNK, elem_size=N,
                    queue_num=0,
                )
                seng = nc.sync if c % 2 == 1 else nc.scalar
                seng.dma_start(
                    out=out_attn[c * CHUNK:(c + 1) * CHUNK, :].rearrange(
                        "(cc p) n -> p cc n", p=128),
                    in_=gt[:, :, :],
                )
            nc.sync.dma_start(out=idxw[16:32, 192:200], in_=wrapped_src[:, 192:200])
            nc.scalar.dma_start(out=idxw[0:16, 192:200], in_=wrapped_src[:, 192:200])
            gtl = gpool.tile([128, 1, N], f32, tag="gtail")
            nc.gpsimd.dma_gather(
                out_ap=gtl[:, :, :], in_ap=attn[:, :], idxs_ap=idxw[:, 192:200],
                num_idxs=128, num_idxs_reg=GROWS - 3072, elem_size=N,
                queue_num=0,
            )
            nc.sync.dma_start(out=out_attn[3072:GROWS, :],
                              in_=gtl[:GROWS - 3072, 0, :])

    nc.compile()
    return nc


_NC_CACHE = None
TRACE = False          # set by test harness to capture an NTFF profile
LAST_RESULT = None     # BassKernelResults of the most recent kernel() call
TRACE_DIR = None


def _get_nc():
    global _NC_CACHE
    if _NC_CACHE is None:
        _NC_CACHE = build_nc()
    return _NC_CACHE


def _install_trace_hooks():
    """Register the NTFF profile hook (missing from this image's antenv)
    and keep artifacts local instead of uploading to a bucket."""
    import types
    if "antenv.axon_hooks" not in sys.modules:
        from trn_agent_boot.trn_boot import _ntff_profile_via_ctypes
        hook = _ntff_profile_via_ctypes("/opt/axon/libaxon_pjrt.so")
        mod = types.ModuleType("antenv.axon_hooks")
        mod.get_axon_ntff_profile_hook = lambda: hook
        mod.set_axon_ntff_profile_hook = lambda h: None
        sys.modules["antenv.axon_hooks"] = mod
    from concourse import bass_utils as BU
    BU.upload_artifacts = lambda tmpdir: tmpdir


def kernel(attn, value, mask, gumbel_u, output_num_tokens):
    from concourse.bass_utils import run_bass_kernel_spmd

    attn = np.ascontiguousarray(np.asarray(attn, dtype=np.float32))
    value = np.ascontiguousarray(np.asarray(value, dtype=np.float32))
    gumbel_u = np.ascontiguousarray(np.asarray(gumbel_u, dtype=np.float32))
    mask_f = np.ascontiguousarray(np.asarray(mask).astype(np.float32))
    assert int(np.asarray(output_num_tokens)) == K

    nc = _get_nc()
    in_maps = [
        {
            "attn": attn[b].reshape(NROWS, N),
            # partition-major permute so each SBUF partition loads contiguously
            "value": np.ascontiguousarray(
                value[b].reshape(96, 128, D).transpose(1, 0, 2)
            ).reshape(128, 96 * D),
            "gumbel": gumbel_u[b],
            "mask": np.ascontiguousarray(mask_f[b].reshape(8, 128).T),
        }
        for b in range(B)
    ]
    kw = {}
    if TRACE:
        import tempfile
        global TRACE_DIR
        TRACE_DIR = tempfile.mkdtemp(prefix="ats_trace_")
        kw = dict(trace=True, tmpdir=TRACE_DIR)
        _install_trace_hooks()
    res = run_bass_kernel_spmd(nc, in_maps, core_ids=list(range(B)), **kw)
    global LAST_RESULT
    LAST_RESULT = res
    results = res.results
    new_attn = np.stack([r["out_attn"].reshape(H, KP1, N) for r in results])
    new_mask = np.stack([r["out_mask"].astype(bool) for r in results])
    ids = np.stack([r["out_ids"].astype(np.int32) for r in results])
    return new_attn, new_mask, ids


# revision 74
# speedup vs baseline: 1.1406x; 1.1406x over previous
"""AdaptiveTokenSampling Trainium2 kernel.

Data-parallel over batch: each of the 8 NeuronCores processes one batch
element end-to-end (per-row gumbel argmax sampling, sort-free dedup via
presence/rank matmuls, dma_gather of the selected attn rows).

Problem shapes (hardcoded): b=8, h=12, n=1024, d=64, k=256.
"""

import sys

for _p in ("/opt/trn_rl_repo", "/root/.axon_site/_ro/trn_rl_repo"):
    if _p not in sys.path:
        sys.path.append(_p)

import numpy as np

B, H, N, D, K = 8, 12, 1024, 64, 256
KP1 = K + 1                     # 257 output tokens (cls + k)
NROWS = H * N                   # 12288 flattened attn rows per core
GROWS = H * KP1                 # 3084 gathered rows per core
GPAD = 3200                     # 25 * 128
EPS = 1e-6
NEG_BIG = -1.0e30
MASK_VAL = -float(np.finfo(np.float32).max) / 2
POLY_THRESH = 1.0 / 32.0
USE_POLY = False   # log1p-polynomial blend near u~1 (exactness insurance)


def build_nc(debug=False, sim=False):
    """Build the single-core Bass graph (same program for all 8 cores)."""
    import concourse.bacc as bacc
    import concourse.bass as bass
    import concourse.mybir as mybir
    from concourse.tile import TileContext
    from concourse.masks import make_identity

    f32 = mybir.dt.float32
    bf16 = mybir.dt.bfloat16
    i32 = mybir.dt.int32
    i16 = mybir.dt.int16
    u8 = mybir.dt.uint8
    Alu = mybir.AluOpType
    Act = mybir.ActivationFunctionType
    X = mybir.AxisListType.X

    # sim: the simulator's SWDGE model locks each DMASW semaphore to one
    # queue, so the sim build keeps every gather on queue 0. Hardware uses
    # 4 queues for 4x descriptor-ring capacity.
    # Multi-queue gathers deadlock intermittently (Tile's DMASW semaphore
    # assignment is queue-agnostic); keep everything on queue 0.
    nc = bacc.Bacc(num_swdge_queues=4)
    _gq = lambda c: 0

    attn = nc.declare_dram_parameter("attn", [NROWS, N], f32, isOutput=False)
    # value is host-permuted to [128, 96*64]: partition-major so each
    # partition's 24KB loads as one contiguous DMA run
    value = nc.declare_dram_parameter("value", [128, 96 * D], f32, isOutput=False)
    gum = nc.declare_dram_parameter("gumbel", [K, N - 1], f32, isOutput=False)
    # mask is host-packed to column layout [128, 8]: mcol[p, j] = mask[128 j + p]
    maskp = nc.declare_dram_parameter("mask", [128, 8], f32, isOutput=False)
    maskp_v = maskp[:, :]
    out_attn = nc.declare_dram_parameter("out_attn", [GROWS, N], f32, isOutput=True)
    out_ids = nc.declare_dram_parameter("out_ids", [KP1], i32, isOutput=True)
    out_mask = nc.declare_dram_parameter("out_mask", [KP1], u8, isOutput=True)
    idr_scratch = nc.dram_tensor("idr_scratch", [GPAD], i16)
    if debug:
        dbg_cnt = nc.declare_dram_parameter("dbg_cnt", [128, 8], f32, isOutput=True)
        dbg_pos = nc.declare_dram_parameter("dbg_pos", [128, 8], f32, isOutput=True)
        dbg_pref = nc.declare_dram_parameter("dbg_pref", [128, 8], f32, isOutput=True)
        dbg_sv = nc.declare_dram_parameter("dbg_sv", [8], f32, isOutput=True)
        dbg_logits = nc.declare_dram_parameter("dbg_logits", [N], f32, isOutput=True)
        dbg_score = nc.declare_dram_parameter("dbg_score", [2, 128, N], f32,
                                              isOutput=True)

    with TileContext(nc) as tc:
        with (
            tc.tile_pool(name="const", bufs=1) as cpool,
            tc.tile_pool(name="vload", bufs=2) as vpool,
            tc.tile_pool(name="work", bufs=1) as wpool,
            tc.tile_pool(name="jt", bufs=2) as jpool,
            tc.tile_pool(name="gather", bufs=3) as gpool,
            tc.tile_pool(name="psA", bufs=1, space="PSUM") as ppool,
            tc.tile_pool(name="psB", bufs=2, space="PSUM") as spool,
        ):
            # ---- constants ----
            identity = cpool.tile([128, 128], f32, tag="identity")
            make_identity(nc, identity[:])
            ones_col = cpool.tile([128, 1], f32, tag="ones_col")
            nc.vector.memset(ones_col[:], 1.0)
            ones_col_bf = cpool.tile([128, 1], bf16, tag="ones_col_bf")
            nc.vector.memset(ones_col_bf[:], 1.0)
            ones_row = cpool.tile([1, 128], f32, tag="ones_row")
            nc.vector.memset(ones_row[:], 1.0)
            eps_col = cpool.tile([128, 1], f32, tag="eps_col")
            nc.vector.memset(eps_col[:], EPS)
            iota_i = cpool.tile([128, N], i32, tag="iota_i")
            nc.gpsimd.iota(iota_i[:], [[1, N]], channel_multiplier=0)
            iota_f = cpool.tile([128, N], f32, tag="iota_f")
            nc.vector.tensor_copy(iota_f[:], iota_i[:])
            offs_i = cpool.tile([12, 1], i32, tag="offs_i")
            nc.gpsimd.iota(offs_i[:], [[0, 1]], channel_multiplier=N)
            offs_f = cpool.tile([12, 1], f32, tag="offs_f")
            nc.vector.tensor_copy(offs_f[:], offs_i[:])
            # negsl[k, n] = -(k >= n), 8x8, for the exclusive-scan matmul
            negsl = cpool.tile([8, 8], f32, tag="negsl")
            nc.gpsimd.memset(negsl[:], -1.0)
            nc.gpsimd.affine_select(
                out=negsl[:], in_=negsl[:], compare_op=Alu.is_ge, fill=0.0,
                base=0, pattern=[[-1, 8]], channel_multiplier=1,
            )
            iota8_i = cpool.tile([8, 128], i32, tag="iota8_i")
            nc.gpsimd.iota(iota8_i[:], [[1, 128]], channel_multiplier=128)
            iota8f = cpool.tile([8, 128], f32, tag="iota8f")
            nc.vector.tensor_copy(iota8f[:], iota8_i[:])

            # ---- gumbel tiles: t2 = Ln(-(log1p-accurate Ln(u+eps)) + eps) ----
            # (independent of logits; runs while value norms load/compute)
            t2_tiles = []
            for jt in range(2):
                gt = jpool.tile([128, N], f32, tag="gt")
                nc.vector.memset(gt[:, 0:1], 0.0)
                geng = nc.scalar if jt == 0 else nc.sync
                geng.dma_start(out=gt[:, 1:N], in_=gum[jt * 128:(jt + 1) * 128, :])
                if USE_POLY:
                    nc.vector.tensor_scalar(out=gt[:], in0=gt[:], scalar1=EPS,
                                            scalar2=None, op0=Alu.add)  # v = u+eps
                    x = jpool.tile([128, N], f32, tag="x")
                    nc.scalar.activation(x[:], gt[:], Act.Copy, bias=-1.0)
                    t1a = jpool.tile([128, N], f32, tag="t1a")
                    nc.scalar.activation(t1a[:], gt[:], Act.Ln)
                    # cubic log1p for |x| < 1/32: x*(1 + x*(-1/2 + x/3))
                    h1 = jpool.tile([128, N], f32, tag="h1")
                    nc.vector.tensor_scalar(out=h1[:], in0=x[:], scalar1=1.0 / 3.0,
                                            scalar2=-0.5, op0=Alu.mult, op1=Alu.add)
                    nc.vector.tensor_tensor(out=h1[:], in0=h1[:], in1=x[:],
                                            op=Alu.mult)
                    nc.scalar.activation(h1[:], h1[:], Act.Copy, bias=1.0)
                    nc.vector.tensor_tensor(out=h1[:], in0=h1[:], in1=x[:],
                                            op=Alu.mult)
                    nc.scalar.activation(x[:], x[:], Act.Abs)
                    selm = jpool.tile([128, N], u8, tag="selm")
                    nc.vector.tensor_scalar(out=selm[:], in0=x[:],
                                            scalar1=POLY_THRESH,
                                            scalar2=None, op0=Alu.is_lt)
                    nc.vector.copy_predicated(out=t1a[:], mask=selm[:], data=h1[:])
                else:
                    t1a = jpool.tile([128, N], f32, tag="t1a")
                    nc.scalar.activation(t1a[:], gt[:], Act.Ln,
                                         bias=eps_col[:, 0:1])  # Ln(u + eps)
                nc.scalar.activation(t1a[:], t1a[:], Act.Ln,
                                     bias=eps_col[:, 0:1], scale=-1.0)  # t2
                t2_tiles.append(t1a)

            # ---- value norms: vnorm[p, c] = ||value row 128c+p||^2, c = 8h+j ----
            vview = value[:, :].rearrange("p (c d) -> p c d", d=D)  # [128,96,64]
            vnorm = wpool.tile([128, 96], f32, tag="vnorm")
            CH = 24
            for cc in range(96 // CH):
                vt = vpool.tile([128, CH, D], f32, tag="vt")
                eng = nc.sync if cc % 2 == 0 else nc.scalar
                eng.dma_start(out=vt[:, :, :], in_=vview[:, cc * CH:(cc + 1) * CH, :])
                sq = vpool.tile([128, CH, D], f32, tag="sq")
                nc.scalar.activation(sq[:, :, :], vt[:, :, :], Act.Square)
                nc.vector.tensor_reduce(
                    out=vnorm[:, cc * CH:(cc + 1) * CH], in_=sq[:, :, :],
                    axis=X, op=Alu.add,
                )
            nc.scalar.activation(vnorm[:], vnorm[:], Act.Sqrt)

            # ---- cls attn row ----
            a0n = wpool.tile([12, N], f32, tag="a0n")
            a0src = attn[:, :].rearrange("(h t) n -> h t n", t=N)[:, 0, :]  # [12, N]
            nc.sync.dma_start(out=a0n[:, :], in_=a0src)
            nc.vector.memset(a0n[:, 0:1], 0.0)  # exclude t=0
            a0T = wpool.tile([128, 96], f32, tag="a0T")
            a0T_v = a0T[:].rearrange("p (h j) -> p h j", h=12)
            for j8 in range(8):
                tp = spool.tile([128, 12], f32, tag="ps_scratch")
                nc.tensor.transpose(out=tp[:], in_=a0n[:, j8 * 128:(j8 + 1) * 128],
                                    identity=identity[:12, :12])
                nc.vector.tensor_copy(a0T_v[:, :, j8], tp[:])

            # ---- cls[p, j] = sum_h a0T * vnorm ----
            nc.vector.tensor_tensor(out=a0T[:], in0=a0T[:], in1=vnorm[:], op=Alu.mult)
            cls = wpool.tile([128, 8], f32, tag="cls")
            nc.vector.tensor_reduce(
                out=cls[:], in_=a0T[:].rearrange("p (h j) -> p j h", h=12),
                axis=X, op=Alu.add,
            )

            # ---- logits in column layout ----
            csum = wpool.tile([128, 1], f32, tag="csum")
            nc.vector.tensor_reduce(out=csum[:], in_=cls[:], axis=X, op=Alu.add)
            tot_ps = spool.tile([1, 1], f32, tag="ps_small")
            nc.tensor.matmul(tot_ps[:], lhsT=csum[:], rhs=ones_col[:, 0:1],
                             start=True, stop=True)
            tot_sb = wpool.tile([1, 1], f32, tag="tot_sb")
            nc.vector.tensor_copy(tot_sb[:], tot_ps[:])
            nc.vector.tensor_scalar(out=tot_sb[:], in0=tot_sb[:], scalar1=EPS,
                                    scalar2=None, op0=Alu.add)
            nc.vector.reciprocal(tot_sb[:], tot_sb[:])
            totb = wpool.tile([128, 1], f32, tag="totb")
            nc.gpsimd.partition_broadcast(totb[:], tot_sb[:])

            lm = wpool.tile([128, 8], f32, tag="lm")
            nc.vector.tensor_scalar(out=lm[:], in0=cls[:], scalar1=totb[:, 0:1],
                                    scalar2=None, op0=Alu.mult)
            nc.scalar.activation(lm[:], lm[:], Act.Ln, bias=eps_col[:, 0:1], scale=1.0)
            mcol = wpool.tile([128, 8], f32, tag="mcol")
            nc.sync.dma_start(out=mcol[:, :], in_=maskp_v)
            im = wpool.tile([128, 8], f32, tag="im")
            nc.vector.tensor_scalar(out=im[:], in0=mcol[:], scalar1=-MASK_VAL,
                                    scalar2=MASK_VAL, op0=Alu.mult, op1=Alu.add)
            nc.vector.tensor_tensor(out=lm[:], in0=lm[:], in1=mcol[:], op=Alu.mult)
            nc.vector.tensor_tensor(out=lm[:], in0=lm[:], in1=im[:], op=Alu.add)
            nc.vector.memset(lm[0:1, 0:1], NEG_BIG)  # t=0 excluded

            # ---- logits -> row -> broadcast over partitions via PE ----
            lrow8 = spool.tile([8, 128], f32, tag="ps_scratch")
            nc.tensor.transpose(out=lrow8[:], in_=lm[:], identity=identity[:])
            lrow8_sb = wpool.tile([8, 128], f32, tag="lrow8_sb")
            nc.vector.tensor_copy(lrow8_sb[:], lrow8[:])
            lrow = wpool.tile([1, N], f32, tag="lrow")
            nc.sync.dma_start(
                out=lrow[0:1, :].rearrange("x (j p) -> x j p", p=128),
                in_=lrow8_sb[:, :],
            )
            lb_ps = ppool.tile([128, N], f32, tag="ps_lb")
            for half in range(2):
                sl = slice(half * 512, (half + 1) * 512)
                nc.tensor.matmul(lb_ps[:, sl], lhsT=ones_row[0:1, :],
                                 rhs=lrow[0:1, sl], start=True, stop=True)

            # ---- scores + per-slot counts (cnt8[j, p] = count for t=128j+p) ----
            cnt8_tiles = []
            for jt in range(2):
                score = jpool.tile([128, N], f32, tag="gt")  # reuse slot rotation
                nc.vector.tensor_tensor(out=score[:], in0=lb_ps[:],
                                        in1=t2_tiles[jt][:], op=Alu.subtract)
                maxv = jpool.tile([128, 1], f32, tag="maxv")
                nc.vector.tensor_reduce(out=maxv[:], in_=score[:], axis=X, op=Alu.max)
                oh = jpool.tile([128, N], bf16, tag="ohbf")
                nc.vector.tensor_scalar(out=oh[:], in0=score[:],
                                        scalar1=maxv[:, 0:1], scalar2=None,
                                        op0=Alu.is_equal)
                if debug:
                    nc.sync.dma_start(out=dbg_score[jt, :, :], in_=score[:, :])
                cnt8_jt = ppool.tile([128, 8], f32, tag=f"ps_cnt8{jt}")
                cnt8_tiles.append(cnt8_jt)
                for j in range(8):
                    nc.tensor.matmul(cnt8_jt[:, j:j + 1],
                                     lhsT=oh[:, j * 128:(j + 1) * 128],
                                     rhs=ones_col_bf[:, 0:1],
                                     start=True, stop=True)

            # ---- presence / rank / position ([128,8] col -> [8,128] scan) ----
            cnt8a = wpool.tile([128, 8], f32, tag="cnt8a")
            nc.vector.tensor_copy(cnt8a[:], cnt8_tiles[0][:])
            nc.vector.tensor_tensor(out=cnt8a[:], in0=cnt8a[:],
                                    in1=cnt8_tiles[1][:], op=Alu.add)
            prescol = wpool.tile([128, 8], f32, tag="prescol")
            nc.vector.tensor_scalar(out=prescol[:], in0=cnt8a[:], scalar1=0.5,
                                    scalar2=None, op0=Alu.is_ge)
            pres8_ps = spool.tile([8, 128], f32, tag="ps_scratch")
            nc.tensor.transpose(out=pres8_ps[:], in_=prescol[:, :],
                                identity=identity[:])
            pres8 = pres8_ps
            scan8 = wpool.tile([8, 128], f32, tag="scan8")
            nc.vector.tensor_tensor_scan(
                out=scan8[:], data0=pres8[:], data1=iota8f[:, :],
                initial=0.0, op0=Alu.add, op1=Alu.bypass,
            )
            # svT[j] = excl[j] - m  via negsl[k, j] = -(k >= j)
            svT_ps = spool.tile([8, 1], f32, tag="ps_small")
            nc.tensor.matmul(svT_ps[:], lhsT=negsl[:8, :8], rhs=scan8[:, 127:128],
                             start=True, stop=True)
            svT = wpool.tile([8, 1], f32, tag="svT")
            nc.vector.tensor_copy(svT[:], svT_ps[:])
            # pos[j, p] = scan8 + (excl - m) + 256 ; mt = t * pres
            pos8 = wpool.tile([8, 128], f32, tag="pos8")
            nc.vector.tensor_scalar(out=pos8[:], in0=scan8[:], scalar1=svT[:, 0:1],
                                    scalar2=float(K), op0=Alu.add, op1=Alu.add)
            mt8 = wpool.tile([8, 128], f32, tag="mt8")
            nc.vector.tensor_tensor(out=mt8[:], in0=iota8f[:, :], in1=pres8[:],
                                    op=Alu.mult)
            pm_ps = spool.tile([128, 16], f32, tag="ps_scratch")
            nc.tensor.transpose(out=pm_ps[:, 0:8], in_=pos8[:, :],
                                identity=identity[:8, :8])
            nc.tensor.transpose(out=pm_ps[:, 8:16], in_=mt8[:, :],
                                identity=identity[:8, :8])
            pm = wpool.tile([128, 16], f32, tag="pm")
            nc.vector.tensor_copy(pm[:], pm_ps[:])
            if debug:
                nc.sync.dma_start(out=dbg_cnt[:, :], in_=pm[:, 0:8])
                nc.sync.dma_start(out=dbg_pos[:, :], in_=pm[:, 0:8])
                nc.sync.dma_start(out=dbg_pref[:, :], in_=pm[:, 8:16])
                nc.sync.dma_start(out=dbg_sv[:], in_=svT[0:1, 0:1])
                nc.sync.dma_start(out=dbg_logits[:], in_=lrow[0:1, :])

            # ---- scatter ids: ids[pos[p,j]] = t(p,j) for present entries ----
            ohp_all = wpool.tile([128, 8, KP1], f32, tag="ohp_all")
            for j in range(8):
                nc.vector.tensor_scalar(out=ohp_all[:, j, :], in0=iota_f[:, :KP1],
                                        scalar1=pm[:, j:j + 1], scalar2=None,
                                        op0=Alu.is_equal)
            ids_ps = spool.tile([1, KP1], f32, tag="ps_small")
            for j in range(8):
                nc.tensor.matmul(ids_ps[:], lhsT=pm[:, 8 + j:9 + j],
                                 rhs=ohp_all[:, j, :],
                                 start=(j == 0), stop=(j == 7))
            ids_f = wpool.tile([1, KP1], f32, tag="ids_f")
            nc.vector.tensor_copy(ids_f[:], ids_ps[:])
            maskf = wpool.tile([1, KP1], f32, tag="maskf")
            nc.vector.tensor_scalar(out=maskf[:], in0=ids_f[:], scalar1=0.5,
                                    scalar2=None, op0=Alu.is_ge)
            nc.vector.memset(maskf[0:1, 0:1], 1.0)
            ids_i = wpool.tile([1, KP1], i32, tag="ids_i")
            nc.vector.tensor_copy(ids_i[:], ids_f[:])
            mask_u = wpool.tile([1, KP1], u8, tag="mask_u")
            nc.vector.tensor_copy(mask_u[:], maskf[:])
            nc.sync.dma_start(out=out_ids[:], in_=ids_i[0:1, :])
            nc.sync.dma_start(out=out_mask[:], in_=mask_u[0:1, :])

            # ---- gather index list: flat = 257h + q -> row 1024h + ids[q] ----
            ids12 = wpool.tile([12, KP1], f32, tag="ids12")
            nc.gpsimd.partition_broadcast(ids12[:], ids_f[0:1, :])
            idr16 = wpool.tile([12, KP1], i16, tag="idr16")
            nc.vector.tensor_scalar(out=idr16[:], in0=ids12[:],
                                    scalar1=offs_f[:, 0:1], scalar2=None, op0=Alu.add)
            nc.sync.dma_start(out=idr_scratch[0:GROWS], in_=idr16[:, :])
            padt = wpool.tile([1, GPAD - GROWS], i16, tag="padt")
            nc.vector.memset(padt[:], -1)
            nc.scalar.dma_start(out=idr_scratch[GROWS:GPAD], in_=padt[0:1, :])
            # wrapped idxs: idxw[p, s] = idr[16 s + p]. dma_gather on queue q
            # reads only partitions [32q+16, 32q+32); also fill [0:16] for the
            # simulator's model.
            idxw = wpool.tile([128, GPAD // 16], i16, tag="idxw")
            nc.gpsimd.memset(idxw[:], 0)
            wrapped_src = idr_scratch[:].rearrange("(s p) -> p s", p=16)

            # ---- gather + store, chunks of 512 rows (+ final 12), queue 0 ----
            CHUNK = 512
            SC = CHUNK // 16
            for c in range(6):
                q = _gq(c)
                w0 = 32 * q + 16
                eng = nc.sync if c % 2 == 0 else nc.scalar
                eng.dma_start(out=idxw[w0:w0 + 16, c * SC:(c + 1) * SC],
                              in_=wrapped_src[:, c * SC:(c + 1) * SC])
                eng.dma_start(out=idxw[0:16, c * SC:(c + 1) * SC],
                              in_=wrapped_src[:, c * SC:(c + 1) * SC])
                gt = gpool.tile([128, CHUNK // 128, N], f32, tag="gchunk")
                nc.gpsimd.dma_gather(
                    out_ap=gt[:, :, :], in_ap=attn[:, :],
                    idxs_ap=idxw[:, c * SC:(c + 1) * SC],
                    num_idxs=CHUNK, num_idxs_reg=CHUNK, elem_size=N,
                    queue_num=q,
                )
                seng = nc.sync if c % 2 == 1 else nc.scalar
                seng.dma_start(
                    out=out_attn[c * CHUNK:(c + 1) * CHUNK, :].rearrange(
                        "(cc p) n -> p cc n", p=128),
                    in_=gt[:, :, :],
                )
            qt = _gq(6)
            wt = 32 * qt + 16
            nc.sync.dma_start(out=idxw[wt:wt + 16, 192:200],
                              in_=wrapped_src[:, 192:200])
            nc.scalar.dma_start(out=idxw[0:16, 192:200], in_=wrapped_src[:, 192:200])
            gtl = gpool.tile([128, 1, N], f32, tag="gtail")
            nc.gpsimd.dma_gather(
                out_ap=gtl[:, :, :], in_ap=attn[:, :], idxs_ap=idxw[:, 192:200],
                num_idxs=128, num_idxs_reg=GROWS - 3072, elem_size=N,
                queue_num=qt,
            )
            nc.sync.dma_start(out=out_attn[3072:GROWS, :],
                              in_=gtl[:GROWS - 3072, 0, :])

    nc.compile()
    return nc


_NC_CACHE = None
TRACE = False          # set by test harness to capture an NTFF profile
LAST_RESULT = None     # BassKernelResults of the most recent kernel() call
TRACE_DIR = None


def _get_nc():
    global _NC_CACHE
    if _NC_CACHE is None:
        _NC_CACHE = build_nc()
    return _NC_CACHE


def _install_trace_hooks():
    """Register the NTFF profile hook (missing from this image's antenv)
    and keep artifacts local instead of uploading to a bucket."""
    import types
    if "antenv.axon_hooks" not in sys.modules:
        from trn_agent_boot.trn_boot import _ntff_profile_via_ctypes
        hook = _ntff_profile_via_ctypes("/opt/axon/libaxon_pjrt.so")
        mod = types.ModuleType("antenv.axon_hooks")
        mod.get_axon_ntff_profile_hook = lambda: hook
        mod.set_axon_ntff_profile_hook = lambda h: None
        sys.modules["antenv.axon_hooks"] = mod
    from concourse import bass_utils as BU
    BU.upload_artifacts = lambda tmpdir: tmpdir


def kernel(attn, value, mask, gumbel_u, output_num_tokens):
    from concourse.bass_utils import run_bass_kernel_spmd

    attn = np.ascontiguousarray(np.asarray(attn, dtype=np.float32))
    value = np.ascontiguousarray(np.asarray(value, dtype=np.float32))
    gumbel_u = np.ascontiguousarray(np.asarray(gumbel_u, dtype=np.float32))
    mask_f = np.ascontiguousarray(np.asarray(mask).astype(np.float32))
    assert int(np.asarray(output_num_tokens)) == K

    nc = _get_nc()
    in_maps = [
        {
            "attn": attn[b].reshape(NROWS, N),
            # partition-major permute so each SBUF partition loads contiguously
            "value": np.ascontiguousarray(
                value[b].reshape(96, 128, D).transpose(1, 0, 2)
            ).reshape(128, 96 * D),
            "gumbel": gumbel_u[b],
            "mask": np.ascontiguousarray(mask_f[b].reshape(8, 128).T),
        }
        for b in range(B)
    ]
    kw = {}
    if TRACE:
        import tempfile
        global TRACE_DIR
        TRACE_DIR = tempfile.mkdtemp(prefix="ats_trace_")
        kw = dict(trace=True, tmpdir=TRACE_DIR)
        _install_trace_hooks()
    res = run_bass_kernel_spmd(nc, in_maps, core_ids=list(range(B)), **kw)
    global LAST_RESULT
    LAST_RESULT = res
    results = res.results
    new_attn = np.stack([r["out_attn"].reshape(H, KP1, N) for r in results])
    new_mask = np.stack([r["out_mask"].astype(bool) for r in results])
    ids = np.stack([r["out_ids"].astype(np.int32) for r in results])
    return new_attn, new_mask, ids


# revision 76
# speedup vs baseline: 1.1454x; 1.0042x over previous
"""AdaptiveTokenSampling Trainium2 kernel.

Data-parallel over batch: each of the 8 NeuronCores processes one batch
element end-to-end (per-row gumbel argmax sampling, sort-free dedup via
presence/rank matmuls, dma_gather of the selected attn rows).

Problem shapes (hardcoded): b=8, h=12, n=1024, d=64, k=256.
"""

import sys

for _p in ("/opt/trn_rl_repo", "/root/.axon_site/_ro/trn_rl_repo"):
    if _p not in sys.path:
        sys.path.append(_p)

import numpy as np

B, H, N, D, K = 8, 12, 1024, 64, 256
KP1 = K + 1                     # 257 output tokens (cls + k)
NROWS = H * N                   # 12288 flattened attn rows per core
GROWS = H * KP1                 # 3084 gathered rows per core
GPAD = 3200                     # 25 * 128
EPS = 1e-6
NEG_BIG = -1.0e30
MASK_VAL = -float(np.finfo(np.float32).max) / 2
POLY_THRESH = 1.0 / 32.0
USE_POLY = False   # log1p-polynomial blend near u~1 (exactness insurance)


def build_nc(debug=False, sim=False):
    """Build the single-core Bass graph (same program for all 8 cores)."""
    import concourse.bacc as bacc
    import concourse.bass as bass
    import concourse.mybir as mybir
    from concourse.tile import TileContext
    from concourse.masks import make_identity

    f32 = mybir.dt.float32
    bf16 = mybir.dt.bfloat16
    i32 = mybir.dt.int32
    i16 = mybir.dt.int16
    u8 = mybir.dt.uint8
    Alu = mybir.AluOpType
    Act = mybir.ActivationFunctionType
    X = mybir.AxisListType.X

    # sim: the simulator's SWDGE model locks each DMASW semaphore to one
    # queue, so the sim build keeps every gather on queue 0. Hardware uses
    # 4 queues for 4x descriptor-ring capacity.
    # Multi-queue gathers deadlock intermittently (Tile's DMASW semaphore
    # assignment is queue-agnostic); keep everything on queue 0.
    nc = bacc.Bacc(num_swdge_queues=1, dynamic_dma_scratch_size=49152)
    _gq = lambda c: 0

    attn = nc.declare_dram_parameter("attn", [NROWS, N], f32, isOutput=False)
    # value is host-permuted to [128, 96*64]: partition-major so each
    # partition's 24KB loads as one contiguous DMA run
    value = nc.declare_dram_parameter("value", [128, 96 * D], f32, isOutput=False)
    gum = nc.declare_dram_parameter("gumbel", [K, N - 1], f32, isOutput=False)
    # mask is host-packed to column layout [128, 8]: mcol[p, j] = mask[128 j + p]
    maskp = nc.declare_dram_parameter("mask", [128, 8], f32, isOutput=False)
    maskp_v = maskp[:, :]
    out_attn = nc.declare_dram_parameter("out_attn", [GROWS, N], f32, isOutput=True)
    out_ids = nc.declare_dram_parameter("out_ids", [KP1], i32, isOutput=True)
    out_mask = nc.declare_dram_parameter("out_mask", [KP1], u8, isOutput=True)
    idr_scratch = nc.dram_tensor("idr_scratch", [GPAD], i16)
    if debug:
        dbg_cnt = nc.declare_dram_parameter("dbg_cnt", [128, 8], f32, isOutput=True)
        dbg_pos = nc.declare_dram_parameter("dbg_pos", [128, 8], f32, isOutput=True)
        dbg_pref = nc.declare_dram_parameter("dbg_pref", [128, 8], f32, isOutput=True)
        dbg_sv = nc.declare_dram_parameter("dbg_sv", [8], f32, isOutput=True)
        dbg_logits = nc.declare_dram_parameter("dbg_logits", [N], f32, isOutput=True)
        dbg_score = nc.declare_dram_parameter("dbg_score", [2, 128, N], f32,
                                              isOutput=True)

    with TileContext(nc) as tc:
        with (
            tc.tile_pool(name="const", bufs=1) as cpool,
            tc.tile_pool(name="vload", bufs=2) as vpool,
            tc.tile_pool(name="work", bufs=1) as wpool,
            tc.tile_pool(name="jt", bufs=2) as jpool,
            tc.tile_pool(name="gather", bufs=4) as gpool,
            tc.tile_pool(name="psA", bufs=1, space="PSUM") as ppool,
            tc.tile_pool(name="psB", bufs=2, space="PSUM") as spool,
        ):
            # ---- constants ----
            identity = cpool.tile([128, 128], f32, tag="identity")
            make_identity(nc, identity[:])
            ones_col = cpool.tile([128, 1], f32, tag="ones_col")
            nc.vector.memset(ones_col[:], 1.0)
            ones_col_bf = cpool.tile([128, 1], bf16, tag="ones_col_bf")
            nc.vector.memset(ones_col_bf[:], 1.0)
            ones_row = cpool.tile([1, 128], f32, tag="ones_row")
            nc.vector.memset(ones_row[:], 1.0)
            eps_col = cpool.tile([128, 1], f32, tag="eps_col")
            nc.vector.memset(eps_col[:], EPS)
            iota_i = cpool.tile([128, N], i32, tag="iota_i")
            nc.gpsimd.iota(iota_i[:], [[1, N]], channel_multiplier=0)
            iota_f = cpool.tile([128, N], f32, tag="iota_f")
            nc.vector.tensor_copy(iota_f[:], iota_i[:])
            offs_i = cpool.tile([12, 1], i32, tag="offs_i")
            nc.gpsimd.iota(offs_i[:], [[0, 1]], channel_multiplier=N)
            offs_f = cpool.tile([12, 1], f32, tag="offs_f")
            nc.vector.tensor_copy(offs_f[:], offs_i[:])
            # negsl[k, n] = -(k >= n), 8x8, for the exclusive-scan matmul
            negsl = cpool.tile([8, 8], f32, tag="negsl")
            nc.gpsimd.memset(negsl[:], -1.0)
            nc.gpsimd.affine_select(
                out=negsl[:], in_=negsl[:], compare_op=Alu.is_ge, fill=0.0,
                base=0, pattern=[[-1, 8]], channel_multiplier=1,
            )
            iota8_i = cpool.tile([8, 128], i32, tag="iota8_i")
            nc.gpsimd.iota(iota8_i[:], [[1, 128]], channel_multiplier=128)
            iota8f = cpool.tile([8, 128], f32, tag="iota8f")
            nc.vector.tensor_copy(iota8f[:], iota8_i[:])

            # ---- gumbel tiles: t2 = Ln(-(log1p-accurate Ln(u+eps)) + eps) ----
            # (independent of logits; runs while value norms load/compute)
            t2_tiles = []
            for jt in range(2):
                gt = jpool.tile([128, N], f32, tag="gt")
                nc.vector.memset(gt[:, 0:1], 0.0)
                geng = nc.scalar if jt == 0 else nc.sync
                geng.dma_start(out=gt[:, 1:N], in_=gum[jt * 128:(jt + 1) * 128, :])
                if USE_POLY:
                    nc.vector.tensor_scalar(out=gt[:], in0=gt[:], scalar1=EPS,
                                            scalar2=None, op0=Alu.add)  # v = u+eps
                    x = jpool.tile([128, N], f32, tag="x")
                    nc.scalar.activation(x[:], gt[:], Act.Copy, bias=-1.0)
                    t1a = jpool.tile([128, N], f32, tag="t1a")
                    nc.scalar.activation(t1a[:], gt[:], Act.Ln)
                    # cubic log1p for |x| < 1/32: x*(1 + x*(-1/2 + x/3))
                    h1 = jpool.tile([128, N], f32, tag="h1")
                    nc.vector.tensor_scalar(out=h1[:], in0=x[:], scalar1=1.0 / 3.0,
                                            scalar2=-0.5, op0=Alu.mult, op1=Alu.add)
                    nc.vector.tensor_tensor(out=h1[:], in0=h1[:], in1=x[:],
                                            op=Alu.mult)
                    nc.scalar.activation(h1[:], h1[:], Act.Copy, bias=1.0)
                    nc.vector.tensor_tensor(out=h1[:], in0=h1[:], in1=x[:],
                                            op=Alu.mult)
                    nc.scalar.activation(x[:], x[:], Act.Abs)
                    selm = jpool.tile([128, N], u8, tag="selm")
                    nc.vector.tensor_scalar(out=selm[:], in0=x[:],
                                            scalar1=POLY_THRESH,
                                            scalar2=None, op0=Alu.is_lt)
                    nc.vector.copy_predicated(out=t1a[:], mask=selm[:], data=h1[:])
                else:
                    t1a = jpool.tile([128, N], f32, tag="t1a")
                    nc.scalar.activation(t1a[:], gt[:], Act.Ln,
                                         bias=eps_col[:, 0:1])  # Ln(u + eps)
                nc.scalar.activation(t1a[:], t1a[:], Act.Ln,
                                     bias=eps_col[:, 0:1], scale=-1.0)  # t2
                t2_tiles.append(t1a)

            # ---- value norms: vnorm[p, c] = ||value row 128c+p||^2, c = 8h+j ----
            vview = value[:, :].rearrange("p (c d) -> p c d", d=D)  # [128,96,64]
            vnorm = wpool.tile([128, 96], f32, tag="vnorm")
            CH = 24
            for cc in range(96 // CH):
                vt = vpool.tile([128, CH, D], f32, tag="vt")
                eng = nc.sync if cc % 2 == 0 else nc.scalar
                eng.dma_start(out=vt[:, :, :], in_=vview[:, cc * CH:(cc + 1) * CH, :])
                sq = vpool.tile([128, CH, D], f32, tag="sq")
                nc.scalar.activation(sq[:, :, :], vt[:, :, :], Act.Square)
                nc.vector.tensor_reduce(
                    out=vnorm[:, cc * CH:(cc + 1) * CH], in_=sq[:, :, :],
                    axis=X, op=Alu.add,
                )
            nc.scalar.activation(vnorm[:], vnorm[:], Act.Sqrt)

            # ---- cls attn row ----
            a0n = wpool.tile([12, N], f32, tag="a0n")
            a0src = attn[:, :].rearrange("(h t) n -> h t n", t=N)[:, 0, :]  # [12, N]
            nc.sync.dma_start(out=a0n[:, :], in_=a0src)
            nc.vector.memset(a0n[:, 0:1], 0.0)  # exclude t=0
            a0T = wpool.tile([128, 96], f32, tag="a0T")
            a0T_v = a0T[:].rearrange("p (h j) -> p h j", h=12)
            for j8 in range(8):
                tp = spool.tile([128, 12], f32, tag="ps_scratch")
                nc.tensor.transpose(out=tp[:], in_=a0n[:, j8 * 128:(j8 + 1) * 128],
                                    identity=identity[:12, :12])
                nc.vector.tensor_copy(a0T_v[:, :, j8], tp[:])

            # ---- cls[p, j] = sum_h a0T * vnorm ----
            nc.vector.tensor_tensor(out=a0T[:], in0=a0T[:], in1=vnorm[:], op=Alu.mult)
            cls = wpool.tile([128, 8], f32, tag="cls")
            nc.vector.tensor_reduce(
                out=cls[:], in_=a0T[:].rearrange("p (h j) -> p j h", h=12),
                axis=X, op=Alu.add,
            )

            # ---- logits in column layout ----
            csum = wpool.tile([128, 1], f32, tag="csum")
            nc.vector.tensor_reduce(out=csum[:], in_=cls[:], axis=X, op=Alu.add)
            tot_ps = spool.tile([1, 1], f32, tag="ps_small")
            nc.tensor.matmul(tot_ps[:], lhsT=csum[:], rhs=ones_col[:, 0:1],
                             start=True, stop=True)
            tot_sb = wpool.tile([1, 1], f32, tag="tot_sb")
            nc.vector.tensor_copy(tot_sb[:], tot_ps[:])
            nc.vector.tensor_scalar(out=tot_sb[:], in0=tot_sb[:], scalar1=EPS,
                                    scalar2=None, op0=Alu.add)
            nc.vector.reciprocal(tot_sb[:], tot_sb[:])
            totb = wpool.tile([128, 1], f32, tag="totb")
            nc.gpsimd.partition_broadcast(totb[:], tot_sb[:])

            lm = wpool.tile([128, 8], f32, tag="lm")
            nc.vector.tensor_scalar(out=lm[:], in0=cls[:], scalar1=totb[:, 0:1],
                                    scalar2=None, op0=Alu.mult)
            nc.scalar.activation(lm[:], lm[:], Act.Ln, bias=eps_col[:, 0:1], scale=1.0)
            mcol = wpool.tile([128, 8], f32, tag="mcol")
            nc.sync.dma_start(out=mcol[:, :], in_=maskp_v)
            im = wpool.tile([128, 8], f32, tag="im")
            nc.vector.tensor_scalar(out=im[:], in0=mcol[:], scalar1=-MASK_VAL,
                                    scalar2=MASK_VAL, op0=Alu.mult, op1=Alu.add)
            nc.vector.tensor_tensor(out=lm[:], in0=lm[:], in1=mcol[:], op=Alu.mult)
            nc.vector.tensor_tensor(out=lm[:], in0=lm[:], in1=im[:], op=Alu.add)
            nc.vector.memset(lm[0:1, 0:1], NEG_BIG)  # t=0 excluded

            # ---- logits -> row -> broadcast over partitions via PE ----
            lrow8 = spool.tile([8, 128], f32, tag="ps_scratch")
            nc.tensor.transpose(out=lrow8[:], in_=lm[:], identity=identity[:])
            lrow8_sb = wpool.tile([8, 128], f32, tag="lrow8_sb")
            nc.vector.tensor_copy(lrow8_sb[:], lrow8[:])
            lrow = wpool.tile([1, N], f32, tag="lrow")
            nc.sync.dma_start(
                out=lrow[0:1, :].rearrange("x (j p) -> x j p", p=128),
                in_=lrow8_sb[:, :],
            )
            lb_ps = ppool.tile([128, N], f32, tag="ps_lb")
            for half in range(2):
                sl = slice(half * 512, (half + 1) * 512)
                nc.tensor.matmul(lb_ps[:, sl], lhsT=ones_row[0:1, :],
                                 rhs=lrow[0:1, sl], start=True, stop=True)

            # ---- scores + per-slot counts (cnt8[j, p] = count for t=128j+p) ----
            cnt8_tiles = []
            for jt in range(2):
                score = jpool.tile([128, N], f32, tag="gt")  # reuse slot rotation
                nc.vector.tensor_tensor(out=score[:], in0=lb_ps[:],
                                        in1=t2_tiles[jt][:], op=Alu.subtract)
                maxv = jpool.tile([128, 1], f32, tag="maxv")
                nc.vector.tensor_reduce(out=maxv[:], in_=score[:], axis=X, op=Alu.max)
                oh = jpool.tile([128, N], bf16, tag="ohbf")
                nc.vector.tensor_scalar(out=oh[:], in0=score[:],
                                        scalar1=maxv[:, 0:1], scalar2=None,
                                        op0=Alu.is_equal)
                if debug:
                    nc.sync.dma_start(out=dbg_score[jt, :, :], in_=score[:, :])
                cnt8_jt = ppool.tile([128, 8], f32, tag=f"ps_cnt8{jt}")
                cnt8_tiles.append(cnt8_jt)
                for j in range(8):
                    nc.tensor.matmul(cnt8_jt[:, j:j + 1],
                                     lhsT=oh[:, j * 128:(j + 1) * 128],
                                     rhs=ones_col_bf[:, 0:1],
                                     start=True, stop=True)

            # ---- presence / rank / position ([128,8] col -> [8,128] scan) ----
            cnt8a = wpool.tile([128, 8], f32, tag="cnt8a")
            nc.vector.tensor_copy(cnt8a[:], cnt8_tiles[0][:])
            nc.vector.tensor_tensor(out=cnt8a[:], in0=cnt8a[:],
                                    in1=cnt8_tiles[1][:], op=Alu.add)
            prescol = wpool.tile([128, 8], f32, tag="prescol")
            nc.vector.tensor_scalar(out=prescol[:], in0=cnt8a[:], scalar1=0.5,
                                    scalar2=None, op0=Alu.is_ge)
            pres8_ps = spool.tile([8, 128], f32, tag="ps_scratch")
            nc.tensor.transpose(out=pres8_ps[:], in_=prescol[:, :],
                                identity=identity[:])
            pres8 = pres8_ps
            scan8 = wpool.tile([8, 128], f32, tag="scan8")
            nc.vector.tensor_tensor_scan(
                out=scan8[:], data0=pres8[:], data1=iota8f[:, :],
                initial=0.0, op0=Alu.add, op1=Alu.bypass,
            )
            # svT[j] = excl[j] - m  via negsl[k, j] = -(k >= j)
            svT_ps = spool.tile([8, 1], f32, tag="ps_small")
            nc.tensor.matmul(svT_ps[:], lhsT=negsl[:8, :8], rhs=scan8[:, 127:128],
                             start=True, stop=True)
            svT = wpool.tile([8, 1], f32, tag="svT")
            nc.vector.tensor_copy(svT[:], svT_ps[:])
            # pos[j, p] = scan8 + (excl - m) + 256 ; mt = t * pres
            pos8 = wpool.tile([8, 128], f32, tag="pos8")
            nc.vector.tensor_scalar(out=pos8[:], in0=scan8[:], scalar1=svT[:, 0:1],
                                    scalar2=float(K), op0=Alu.add, op1=Alu.add)
            mt8 = wpool.tile([8, 128], f32, tag="mt8")
            nc.vector.tensor_tensor(out=mt8[:], in0=iota8f[:, :], in1=pres8[:],
                                    op=Alu.mult)
            pm_ps = spool.tile([128, 16], f32, tag="ps_scratch")
            nc.tensor.transpose(out=pm_ps[:, 0:8], in_=pos8[:, :],
                                identity=identity[:8, :8])
            nc.tensor.transpose(out=pm_ps[:, 8:16], in_=mt8[:, :],
                                identity=identity[:8, :8])
            pm = wpool.tile([128, 16], f32, tag="pm")
            nc.vector.tensor_copy(pm[:], pm_ps[:])
            if debug:
                nc.sync.dma_start(out=dbg_cnt[:, :], in_=pm[:, 0:8])
                nc.sync.dma_start(out=dbg_pos[:, :], in_=pm[:, 0:8])
                nc.sync.dma_start(out=dbg_pref[:, :], in_=pm[:, 8:16])
                nc.sync.dma_start(out=dbg_sv[:], in_=svT[0:1, 0:1])
                nc.sync.dma_start(out=dbg_logits[:], in_=lrow[0:1, :])

            # ---- scatter ids: ids[pos[p,j]] = t(p,j) for present entries ----
            ohp_all = wpool.tile([128, 8, KP1], f32, tag="ohp_all")
            for j in range(8):
                nc.vector.tensor_scalar(out=ohp_all[:, j, :], in0=iota_f[:, :KP1],
                                        scalar1=pm[:, j:j + 1], scalar2=None,
                                        op0=Alu.is_equal)
            ids_ps = spool.tile([1, KP1], f32, tag="ps_small")
            for j in range(8):
                nc.tensor.matmul(ids_ps[:], lhsT=pm[:, 8 + j:9 + j],
                                 rhs=ohp_all[:, j, :],
                                 start=(j == 0), stop=(j == 7))
            ids_f = wpool.tile([1, KP1], f32, tag="ids_f")
            nc.vector.tensor_copy(ids_f[:], ids_ps[:])
            maskf = wpool.tile([1, KP1], f32, tag="maskf")
            nc.vector.tensor_scalar(out=maskf[:], in0=ids_f[:], scalar1=0.5,
                                    scalar2=None, op0=Alu.is_ge)
            nc.vector.memset(maskf[0:1, 0:1], 1.0)
            ids_i = wpool.tile([1, KP1], i32, tag="ids_i")
            nc.vector.tensor_copy(ids_i[:], ids_f[:])
            mask_u = wpool.tile([1, KP1], u8, tag="mask_u")
            nc.vector.tensor_copy(mask_u[:], maskf[:])
            nc.sync.dma_start(out=out_ids[:], in_=ids_i[0:1, :])
            nc.sync.dma_start(out=out_mask[:], in_=mask_u[0:1, :])

            # ---- gather index list: flat = 257h + q -> row 1024h + ids[q] ----
            ids12 = wpool.tile([12, KP1], f32, tag="ids12")
            nc.gpsimd.partition_broadcast(ids12[:], ids_f[0:1, :])
            idr16 = wpool.tile([12, KP1], i16, tag="idr16")
            nc.vector.tensor_scalar(out=idr16[:], in0=ids12[:],
                                    scalar1=offs_f[:, 0:1], scalar2=None, op0=Alu.add)
            nc.sync.dma_start(out=idr_scratch[0:GROWS], in_=idr16[:, :])
            padt = wpool.tile([1, GPAD - GROWS], i16, tag="padt")
            nc.vector.memset(padt[:], -1)
            nc.scalar.dma_start(out=idr_scratch[GROWS:GPAD], in_=padt[0:1, :])
            # wrapped idxs: idxw[p, s] = idr[16 s + p]. dma_gather on queue q
            # reads only partitions [32q+16, 32q+32); also fill [0:16] for the
            # simulator's model.
            idxw = wpool.tile([128, GPAD // 16], i16, tag="idxw")
            nc.gpsimd.memset(idxw[:], 0)
            wrapped_src = idr_scratch[:].rearrange("(s p) -> p s", p=16)

            # ---- gather + store, chunks of 512 rows (+ final 12), queue 0 ----
            CHUNK = 512
            SC = CHUNK // 16
            for c in range(6):
                q = _gq(c)
                w0 = 32 * q + 16
                eng = nc.sync if c % 2 == 0 else nc.scalar
                eng.dma_start(out=idxw[w0:w0 + 16, c * SC:(c + 1) * SC],
                              in_=wrapped_src[:, c * SC:(c + 1) * SC])
                eng.dma_start(out=idxw[0:16, c * SC:(c + 1) * SC],
                              in_=wrapped_src[:, c * SC:(c + 1) * SC])
                gt = gpool.tile([128, CHUNK // 128, N], f32, tag="gchunk")
                nc.gpsimd.dma_gather(
                    out_ap=gt[:, :, :], in_ap=attn[:, :],
                    idxs_ap=idxw[:, c * SC:(c + 1) * SC],
                    num_idxs=CHUNK, num_idxs_reg=CHUNK, elem_size=N,
                    queue_num=q,
                )
                seng = nc.sync if c % 2 == 1 else nc.scalar
                seng.dma_start(
                    out=out_attn[c * CHUNK:(c + 1) * CHUNK, :].rearrange(
                        "(cc p) n -> p cc n", p=128),
                    in_=gt[:, :, :],
                )
            qt = _gq(6)
            wt = 32 * qt + 16
            nc.sync.dma_start(out=idxw[wt:wt + 16, 192:200],
                              in_=wrapped_src[:, 192:200])
            nc.scalar.dma_start(out=idxw[0:16, 192:200], in_=wrapped_src[:, 192:200])
            gtl = gpool.tile([128, 1, N], f32, tag="gtail")
            nc.gpsimd.dma_gather(
                out_ap=gtl[:, :, :], in_ap=attn[:, :], idxs_ap=idxw[:, 192:200],
                num_idxs=128, num_idxs_reg=GROWS - 3072, elem_size=N,
                queue_num=qt,
            )
            nc.sync.dma_start(out=out_attn[3072:GROWS, :],
                              in_=gtl[:GROWS - 3072, 0, :])

    nc.compile()
    return nc


_NC_CACHE = None
TRACE = False          # set by test harness to capture an NTFF profile
LAST_RESULT = None     # BassKernelResults of the most recent kernel() call
TRACE_DIR = None


def _get_nc():
    global _NC_CACHE
    if _NC_CACHE is None:
        _NC_CACHE = build_nc()
    return _NC_CACHE


def _install_trace_hooks():
    """Register the NTFF profile hook (missing from this image's antenv)
    and keep artifacts local instead of uploading to a bucket."""
    import types
    if "antenv.axon_hooks" not in sys.modules:
        from trn_agent_boot.trn_boot import _ntff_profile_via_ctypes
        hook = _ntff_profile_via_ctypes("/opt/axon/libaxon_pjrt.so")
        mod = types.ModuleType("antenv.axon_hooks")
        mod.get_axon_ntff_profile_hook = lambda: hook
        mod.set_axon_ntff_profile_hook = lambda h: None
        sys.modules["antenv.axon_hooks"] = mod
    from concourse import bass_utils as BU
    BU.upload_artifacts = lambda tmpdir: tmpdir


def kernel(attn, value, mask, gumbel_u, output_num_tokens):
    from concourse.bass_utils import run_bass_kernel_spmd

    attn = np.ascontiguousarray(np.asarray(attn, dtype=np.float32))
    value = np.ascontiguousarray(np.asarray(value, dtype=np.float32))
    gumbel_u = np.ascontiguousarray(np.asarray(gumbel_u, dtype=np.float32))
    mask_f = np.ascontiguousarray(np.asarray(mask).astype(np.float32))
    assert int(np.asarray(output_num_tokens)) == K

    nc = _get_nc()
    in_maps = [
        {
            "attn": attn[b].reshape(NROWS, N),
            # partition-major permute so each SBUF partition loads contiguously
            "value": np.ascontiguousarray(
                value[b].reshape(96, 128, D).transpose(1, 0, 2)
            ).reshape(128, 96 * D),
            "gumbel": gumbel_u[b],
            "mask": np.ascontiguousarray(mask_f[b].reshape(8, 128).T),
        }
        for b in range(B)
    ]
    kw = {}
    if TRACE:
        import tempfile
        global TRACE_DIR
        TRACE_DIR = tempfile.mkdtemp(prefix="ats_trace_")
        kw = dict(trace=True, tmpdir=TRACE_DIR)
        _install_trace_hooks()
    res = run_bass_kernel_spmd(nc, in_maps, core_ids=list(range(B)), **kw)
    global LAST_RESULT
    LAST_RESULT = res
    results = res.results
    new_attn = np.stack([r["out_attn"].reshape(H, KP1, N) for r in results])
    new_mask = np.stack([r["out_mask"].astype(bool) for r in results])
    ids = np.stack([r["out_ids"].astype(np.int32) for r in results])
    return new_attn, new_mask, ids


# revision 77
# speedup vs baseline: 1.2265x; 1.0708x over previous
"""AdaptiveTokenSampling Trainium2 kernel.

Data-parallel over batch: each of the 8 NeuronCores processes one batch
element end-to-end (per-row gumbel argmax sampling, sort-free dedup via
presence/rank matmuls, dma_gather of the selected attn rows).

Problem shapes (hardcoded): b=8, h=12, n=1024, d=64, k=256.
"""

import sys

for _p in ("/opt/trn_rl_repo", "/root/.axon_site/_ro/trn_rl_repo"):
    if _p not in sys.path:
        sys.path.append(_p)

import numpy as np

B, H, N, D, K = 8, 12, 1024, 64, 256
KP1 = K + 1                     # 257 output tokens (cls + k)
NROWS = H * N                   # 12288 flattened attn rows per core
GROWS = H * KP1                 # 3084 gathered rows per core
GPAD = 3200                     # 25 * 128
EPS = 1e-6
NEG_BIG = -1.0e30
MASK_VAL = -float(np.finfo(np.float32).max) / 2
POLY_THRESH = 1.0 / 32.0
USE_POLY = False   # log1p-polynomial blend near u~1 (exactness insurance)


def build_nc(debug=False, sim=False):
    """Build the single-core Bass graph (same program for all 8 cores)."""
    import concourse.bacc as bacc
    import concourse.bass as bass
    import concourse.mybir as mybir
    from concourse.tile import TileContext
    from concourse.masks import make_identity

    f32 = mybir.dt.float32
    bf16 = mybir.dt.bfloat16
    i32 = mybir.dt.int32
    i16 = mybir.dt.int16
    u8 = mybir.dt.uint8
    Alu = mybir.AluOpType
    Act = mybir.ActivationFunctionType
    X = mybir.AxisListType.X

    # sim: the simulator's SWDGE model locks each DMASW semaphore to one
    # queue, so the sim build keeps every gather on queue 0. Hardware uses
    # 4 queues for 4x descriptor-ring capacity.
    # Multi-queue gathers deadlock intermittently (Tile's DMASW semaphore
    # assignment is queue-agnostic); keep everything on queue 0.
    nc = bacc.Bacc(num_swdge_queues=1, dynamic_dma_scratch_size=49152)
    _gq = lambda c: 0

    attn = nc.declare_dram_parameter("attn", [NROWS, N], f32, isOutput=False)
    # value is host-permuted to [128, 96*64]: partition-major so each
    # partition's 24KB loads as one contiguous DMA run
    value = nc.declare_dram_parameter("value", [128, 96 * D], f32, isOutput=False)
    gum = nc.declare_dram_parameter("gumbel", [K, N - 1], f32, isOutput=False)
    # mask is host-packed to column layout [128, 8]: mcol[p, j] = mask[128 j + p]
    maskp = nc.declare_dram_parameter("mask", [128, 8], f32, isOutput=False)
    maskp_v = maskp[:, :]
    out_attn = nc.declare_dram_parameter("out_attn", [GROWS, N], f32, isOutput=True)
    out_ids = nc.declare_dram_parameter("out_ids", [KP1], i32, isOutput=True)
    out_mask = nc.declare_dram_parameter("out_mask", [KP1], u8, isOutput=True)
    idr_scratch = nc.dram_tensor("idr_scratch", [GPAD], i16)
    if debug:
        dbg_cnt = nc.declare_dram_parameter("dbg_cnt", [128, 8], f32, isOutput=True)
        dbg_pos = nc.declare_dram_parameter("dbg_pos", [128, 8], f32, isOutput=True)
        dbg_pref = nc.declare_dram_parameter("dbg_pref", [128, 8], f32, isOutput=True)
        dbg_sv = nc.declare_dram_parameter("dbg_sv", [8], f32, isOutput=True)
        dbg_logits = nc.declare_dram_parameter("dbg_logits", [N], f32, isOutput=True)
        dbg_score = nc.declare_dram_parameter("dbg_score", [2, 128, N], f32,
                                              isOutput=True)

    with TileContext(nc) as tc:
        with (
            tc.tile_pool(name="const", bufs=1) as cpool,
            tc.tile_pool(name="vload", bufs=2) as vpool,
            tc.tile_pool(name="work", bufs=1) as wpool,
            tc.tile_pool(name="jt", bufs=2) as jpool,
            tc.tile_pool(name="gather", bufs=4) as gpool,
            tc.tile_pool(name="psA", bufs=1, space="PSUM") as ppool,
            tc.tile_pool(name="psB", bufs=2, space="PSUM") as spool,
        ):
            # ---- constants ----
            identity = cpool.tile([128, 128], f32, tag="identity")
            make_identity(nc, identity[:])
            ones_col = cpool.tile([128, 1], f32, tag="ones_col")
            nc.vector.memset(ones_col[:], 1.0)
            ones_col_bf = cpool.tile([128, 1], bf16, tag="ones_col_bf")
            nc.vector.memset(ones_col_bf[:], 1.0)
            ones_row = cpool.tile([1, 128], f32, tag="ones_row")
            nc.vector.memset(ones_row[:], 1.0)
            eps_col = cpool.tile([128, 1], f32, tag="eps_col")
            nc.vector.memset(eps_col[:], EPS)
            iota_i = cpool.tile([128, N], i32, tag="iota_i")
            nc.gpsimd.iota(iota_i[:], [[1, N]], channel_multiplier=0)
            iota_f = cpool.tile([128, N], f32, tag="iota_f")
            nc.vector.tensor_copy(iota_f[:], iota_i[:])
            offs_i = cpool.tile([12, 1], i32, tag="offs_i")
            nc.gpsimd.iota(offs_i[:], [[0, 1]], channel_multiplier=N)
            offs_f = cpool.tile([12, 1], f32, tag="offs_f")
            nc.vector.tensor_copy(offs_f[:], offs_i[:])
            # negsl[k, n] = -(k >= n), 8x8, for the exclusive-scan matmul
            negsl = cpool.tile([8, 8], f32, tag="negsl")
            nc.gpsimd.memset(negsl[:], -1.0)
            nc.gpsimd.affine_select(
                out=negsl[:], in_=negsl[:], compare_op=Alu.is_ge, fill=0.0,
                base=0, pattern=[[-1, 8]], channel_multiplier=1,
            )
            iota8_i = cpool.tile([8, 128], i32, tag="iota8_i")
            nc.gpsimd.iota(iota8_i[:], [[1, 128]], channel_multiplier=128)
            iota8f = cpool.tile([8, 128], f32, tag="iota8f")
            nc.vector.tensor_copy(iota8f[:], iota8_i[:])

            # ---- gumbel tiles: t2 = Ln(-(log1p-accurate Ln(u+eps)) + eps) ----
            # (independent of logits; runs while value norms load/compute)
            t2_tiles = []
            for jt in range(2):
                gt = jpool.tile([128, N], f32, tag="gt")
                nc.vector.memset(gt[:, 0:1], 0.0)
                geng = nc.scalar if jt == 0 else nc.sync
                geng.dma_start(out=gt[:, 1:N], in_=gum[jt * 128:(jt + 1) * 128, :])
                if USE_POLY:
                    nc.vector.tensor_scalar(out=gt[:], in0=gt[:], scalar1=EPS,
                                            scalar2=None, op0=Alu.add)  # v = u+eps
                    x = jpool.tile([128, N], f32, tag="x")
                    nc.scalar.activation(x[:], gt[:], Act.Copy, bias=-1.0)
                    t1a = jpool.tile([128, N], f32, tag="t1a")
                    nc.scalar.activation(t1a[:], gt[:], Act.Ln)
                    # cubic log1p for |x| < 1/32: x*(1 + x*(-1/2 + x/3))
                    h1 = jpool.tile([128, N], f32, tag="h1")
                    nc.vector.tensor_scalar(out=h1[:], in0=x[:], scalar1=1.0 / 3.0,
                                            scalar2=-0.5, op0=Alu.mult, op1=Alu.add)
                    nc.vector.tensor_tensor(out=h1[:], in0=h1[:], in1=x[:],
                                            op=Alu.mult)
                    nc.scalar.activation(h1[:], h1[:], Act.Copy, bias=1.0)
                    nc.vector.tensor_tensor(out=h1[:], in0=h1[:], in1=x[:],
                                            op=Alu.mult)
                    nc.scalar.activation(x[:], x[:], Act.Abs)
                    selm = jpool.tile([128, N], u8, tag="selm")
                    nc.vector.tensor_scalar(out=selm[:], in0=x[:],
                                            scalar1=POLY_THRESH,
                                            scalar2=None, op0=Alu.is_lt)
                    nc.vector.copy_predicated(out=t1a[:], mask=selm[:], data=h1[:])
                else:
                    t1a = jpool.tile([128, N], f32, tag="t1a")
                    nc.scalar.activation(t1a[:], gt[:], Act.Ln,
                                         bias=eps_col[:, 0:1])  # Ln(u + eps)
                nc.scalar.activation(t1a[:], t1a[:], Act.Ln,
                                     bias=eps_col[:, 0:1], scale=-1.0)  # t2
                t2_tiles.append(t1a)

            # ---- value norms: vnorm[p, c] = ||value row 128c+p||^2, c = 8h+j ----
            vview = value[:, :].rearrange("p (c d) -> p c d", d=D)  # [128,96,64]
            vnorm = wpool.tile([128, 96], f32, tag="vnorm")
            CH = 24
            for cc in range(96 // CH):
                vt = vpool.tile([128, CH, D], f32, tag="vt")
                eng = nc.sync if cc % 2 == 0 else nc.scalar
                eng.dma_start(out=vt[:, :, :], in_=vview[:, cc * CH:(cc + 1) * CH, :])
                sq = vpool.tile([128, CH, D], f32, tag="sq")
                nc.scalar.activation(sq[:, :, :], vt[:, :, :], Act.Square)
                nc.vector.tensor_reduce(
                    out=vnorm[:, cc * CH:(cc + 1) * CH], in_=sq[:, :, :],
                    axis=X, op=Alu.add,
                )
            nc.scalar.activation(vnorm[:], vnorm[:], Act.Sqrt)

            # ---- cls attn row ----
            a0n = wpool.tile([12, N], f32, tag="a0n")
            a0src = attn[:, :].rearrange("(h t) n -> h t n", t=N)[:, 0, :]  # [12, N]
            nc.sync.dma_start(out=a0n[:, :], in_=a0src)
            nc.vector.memset(a0n[:, 0:1], 0.0)  # exclude t=0
            a0T = wpool.tile([128, 96], f32, tag="a0T")
            a0T_v = a0T[:].rearrange("p (h j) -> p h j", h=12)
            for j8 in range(8):
                tp = spool.tile([128, 12], f32, tag="ps_scratch")
                nc.tensor.transpose(out=tp[:], in_=a0n[:, j8 * 128:(j8 + 1) * 128],
                                    identity=identity[:12, :12])
                nc.vector.tensor_copy(a0T_v[:, :, j8], tp[:])

            # ---- cls[p, j] = sum_h a0T * vnorm ----
            nc.vector.tensor_tensor(out=a0T[:], in0=a0T[:], in1=vnorm[:], op=Alu.mult)
            cls = wpool.tile([128, 8], f32, tag="cls")
            nc.vector.tensor_reduce(
                out=cls[:], in_=a0T[:].rearrange("p (h j) -> p j h", h=12),
                axis=X, op=Alu.add,
            )

            # ---- logits in column layout ----
            csum = wpool.tile([128, 1], f32, tag="csum")
            nc.vector.tensor_reduce(out=csum[:], in_=cls[:], axis=X, op=Alu.add)
            tot_ps = spool.tile([1, 1], f32, tag="ps_small")
            nc.tensor.matmul(tot_ps[:], lhsT=csum[:], rhs=ones_col[:, 0:1],
                             start=True, stop=True)
            tot_sb = wpool.tile([1, 1], f32, tag="tot_sb")
            nc.vector.tensor_copy(tot_sb[:], tot_ps[:])
            nc.vector.tensor_scalar(out=tot_sb[:], in0=tot_sb[:], scalar1=EPS,
                                    scalar2=None, op0=Alu.add)
            nc.vector.reciprocal(tot_sb[:], tot_sb[:])
            totb = wpool.tile([128, 1], f32, tag="totb")
            nc.gpsimd.partition_broadcast(totb[:], tot_sb[:])

            lm = wpool.tile([128, 8], f32, tag="lm")
            nc.vector.tensor_scalar(out=lm[:], in0=cls[:], scalar1=totb[:, 0:1],
                                    scalar2=None, op0=Alu.mult)
            nc.scalar.activation(lm[:], lm[:], Act.Ln, bias=eps_col[:, 0:1], scale=1.0)
            mcol = wpool.tile([128, 8], f32, tag="mcol")
            nc.sync.dma_start(out=mcol[:, :], in_=maskp_v)
            im = wpool.tile([128, 8], f32, tag="im")
            nc.vector.tensor_scalar(out=im[:], in0=mcol[:], scalar1=-MASK_VAL,
                                    scalar2=MASK_VAL, op0=Alu.mult, op1=Alu.add)
            nc.vector.tensor_tensor(out=lm[:], in0=lm[:], in1=mcol[:], op=Alu.mult)
            nc.vector.tensor_tensor(out=lm[:], in0=lm[:], in1=im[:], op=Alu.add)
            nc.vector.memset(lm[0:1, 0:1], NEG_BIG)  # t=0 excluded

            # ---- logits -> row -> broadcast over partitions via PE ----
            lrow8 = spool.tile([8, 128], f32, tag="ps_scratch")
            nc.tensor.transpose(out=lrow8[:], in_=lm[:], identity=identity[:])
            lrow8_sb = wpool.tile([8, 128], f32, tag="lrow8_sb")
            nc.vector.tensor_copy(lrow8_sb[:], lrow8[:])
            lrow = wpool.tile([1, N], f32, tag="lrow")
            nc.sync.dma_start(
                out=lrow[0:1, :].rearrange("x (j p) -> x j p", p=128),
                in_=lrow8_sb[:, :],
            )
            lb_ps = ppool.tile([128, N], f32, tag="ps_lb")
            for half in range(2):
                sl = slice(half * 512, (half + 1) * 512)
                nc.tensor.matmul(lb_ps[:, sl], lhsT=ones_row[0:1, :],
                                 rhs=lrow[0:1, sl], start=True, stop=True)

            # ---- scores + per-slot counts (cnt8[j, p] = count for t=128j+p) ----
            cnt8_tiles = []
            for jt in range(2):
                score = jpool.tile([128, N], f32, tag="gt")  # reuse slot rotation
                nc.vector.tensor_tensor(out=score[:], in0=lb_ps[:],
                                        in1=t2_tiles[jt][:], op=Alu.subtract)
                maxv = jpool.tile([128, 1], f32, tag="maxv")
                nc.vector.tensor_reduce(out=maxv[:], in_=score[:], axis=X, op=Alu.max)
                oh = jpool.tile([128, N], bf16, tag="ohbf")
                nc.vector.tensor_scalar(out=oh[:], in0=score[:],
                                        scalar1=maxv[:, 0:1], scalar2=None,
                                        op0=Alu.is_equal)
                if debug:
                    nc.sync.dma_start(out=dbg_score[jt, :, :], in_=score[:, :])
                cnt8_jt = ppool.tile([128, 8], f32, tag=f"ps_cnt8{jt}")
                cnt8_tiles.append(cnt8_jt)
                for j in range(8):
                    nc.tensor.matmul(cnt8_jt[:, j:j + 1],
                                     lhsT=oh[:, j * 128:(j + 1) * 128],
                                     rhs=ones_col_bf[:, 0:1],
                                     start=True, stop=True)

            # ---- presence / rank / position ([128,8] col -> [8,128] scan) ----
            cnt8a = wpool.tile([128, 8], f32, tag="cnt8a")
            nc.vector.tensor_copy(cnt8a[:], cnt8_tiles[0][:])
            nc.vector.tensor_tensor(out=cnt8a[:], in0=cnt8a[:],
                                    in1=cnt8_tiles[1][:], op=Alu.add)
            prescol = wpool.tile([128, 8], f32, tag="prescol")
            nc.vector.tensor_scalar(out=prescol[:], in0=cnt8a[:], scalar1=0.5,
                                    scalar2=None, op0=Alu.is_ge)
            pres8_ps = spool.tile([8, 128], f32, tag="ps_scratch")
            nc.tensor.transpose(out=pres8_ps[:], in_=prescol[:, :],
                                identity=identity[:])
            pres8 = pres8_ps
            scan8 = wpool.tile([8, 128], f32, tag="scan8")
            nc.vector.tensor_tensor_scan(
                out=scan8[:], data0=pres8[:], data1=iota8f[:, :],
                initial=0.0, op0=Alu.add, op1=Alu.bypass,
            )
            # svT[j] = excl[j] - m  via negsl[k, j] = -(k >= j)
            svT_ps = spool.tile([8, 1], f32, tag="ps_small")
            nc.tensor.matmul(svT_ps[:], lhsT=negsl[:8, :8], rhs=scan8[:, 127:128],
                             start=True, stop=True)
            # pos[j, p] = scan8 + (excl - m) + 256 ; mt = t * pres
            pos8 = wpool.tile([8, 128], f32, tag="pos8")
            nc.vector.tensor_scalar(out=pos8[:], in0=scan8[:],
                                    scalar1=svT_ps[:, 0:1],
                                    scalar2=float(K), op0=Alu.add, op1=Alu.add)
            mt8 = wpool.tile([8, 128], f32, tag="mt8")
            nc.vector.tensor_tensor(out=mt8[:], in0=iota8f[:, :], in1=pres8[:],
                                    op=Alu.mult)
            pm_ps = spool.tile([128, 16], f32, tag="ps_scratch")
            nc.tensor.transpose(out=pm_ps[:, 0:8], in_=pos8[:, :],
                                identity=identity[:8, :8])
            nc.tensor.transpose(out=pm_ps[:, 8:16], in_=mt8[:, :],
                                identity=identity[:8, :8])
            pmm = wpool.tile([128, 8], f32, tag="pmm")
            nc.vector.tensor_copy(pmm[:], pm_ps[:, 8:16])
            if debug:
                nc.sync.dma_start(out=dbg_pos[:, :], in_=pmm[:, :])
                nc.sync.dma_start(out=dbg_pref[:, :], in_=pmm[:, :])
                nc.sync.dma_start(out=dbg_logits[:], in_=lrow[0:1, :])

            # ---- scatter ids (12-head replicated): idr[h, pos] = t + 1024 h ----
            ohp_all = wpool.tile([128, 8, KP1], f32, tag="ohp_all")
            for j in range(8):
                nc.vector.tensor_scalar(out=ohp_all[:, j, :], in0=iota_f[:, :KP1],
                                        scalar1=pm_ps[:, j:j + 1], scalar2=None,
                                        op0=Alu.is_equal)
            ids12_ps = spool.tile([12, KP1], f32, tag="ps_small")
            for j in range(8):
                nc.tensor.matmul(ids12_ps[:], lhsT=pmm[:, j:j + 1].to_broadcast(
                                     [128, 12]),
                                 rhs=ohp_all[:, j, :],
                                 start=(j == 0), stop=(j == 7))
            idr16 = wpool.tile([12, KP1], i16, tag="idr16")
            nc.vector.tensor_scalar(out=idr16[:], in0=ids12_ps[:],
                                    scalar1=offs_f[:, 0:1], scalar2=None, op0=Alu.add)
            nc.sync.dma_start(out=idr_scratch[0:GROWS], in_=idr16[:, :])
            # ids / mask outputs (off the gather critical path)
            ids_f = wpool.tile([1, KP1], f32, tag="ids_f")
            nc.vector.tensor_copy(ids_f[:], ids12_ps[0:1, :])
            maskf = wpool.tile([1, KP1], f32, tag="maskf")
            nc.vector.tensor_scalar(out=maskf[:], in0=ids_f[:], scalar1=0.5,
                                    scalar2=None, op0=Alu.is_ge)
            nc.vector.memset(maskf[0:1, 0:1], 1.0)
            ids_i = wpool.tile([1, KP1], i32, tag="ids_i")
            nc.vector.tensor_copy(ids_i[:], ids_f[:])
            mask_u = wpool.tile([1, KP1], u8, tag="mask_u")
            nc.vector.tensor_copy(mask_u[:], maskf[:])
            nc.sync.dma_start(out=out_ids[:], in_=ids_i[0:1, :])
            nc.sync.dma_start(out=out_mask[:], in_=mask_u[0:1, :])
            padt = wpool.tile([1, GPAD - GROWS], i16, tag="padt")
            nc.vector.memset(padt[:], -1)
            nc.scalar.dma_start(out=idr_scratch[GROWS:GPAD], in_=padt[0:1, :])
            # wrapped idxs: idxw[p, s] = idr[16 s + p]. dma_gather on queue q
            # reads only partitions [32q+16, 32q+32); also fill [0:16] for the
            # simulator's model.
            idxw = wpool.tile([128, GPAD // 16], i16, tag="idxw")
            nc.gpsimd.memset(idxw[:], 0)
            wrapped_src = idr_scratch[:].rearrange("(s p) -> p s", p=16)

            # ---- gather + store, chunks of 512 rows (+ final 12), queue 0 ----
            CHUNK = 512
            SC = CHUNK // 16
            for c in range(6):
                q = _gq(c)
                w0 = 32 * q + 16
                eng = nc.sync if c % 2 == 0 else nc.scalar
                eng.dma_start(out=idxw[w0:w0 + 16, c * SC:(c + 1) * SC],
                              in_=wrapped_src[:, c * SC:(c + 1) * SC])
                eng.dma_start(out=idxw[0:16, c * SC:(c + 1) * SC],
                              in_=wrapped_src[:, c * SC:(c + 1) * SC])
                gt = gpool.tile([128, CHUNK // 128, N], f32, tag="gchunk")
                nc.gpsimd.dma_gather(
                    out_ap=gt[:, :, :], in_ap=attn[:, :],
                    idxs_ap=idxw[:, c * SC:(c + 1) * SC],
                    num_idxs=CHUNK, num_idxs_reg=CHUNK, elem_size=N,
                    queue_num=q,
                )
                seng = nc.sync if c % 2 == 1 else nc.scalar
                seng.dma_start(
                    out=out_attn[c * CHUNK:(c + 1) * CHUNK, :].rearrange(
                        "(cc p) n -> p cc n", p=128),
                    in_=gt[:, :, :],
                )
            qt = _gq(6)
            wt = 32 * qt + 16
            nc.sync.dma_start(out=idxw[wt:wt + 16, 192:200],
                              in_=wrapped_src[:, 192:200])
            nc.scalar.dma_start(out=idxw[0:16, 192:200], in_=wrapped_src[:, 192:200])
            gtl = gpool.tile([128, 1, N], f32, tag="gtail")
            nc.gpsimd.dma_gather(
                out_ap=gtl[:, :, :], in_ap=attn[:, :], idxs_ap=idxw[:, 192:200],
                num_idxs=128, num_idxs_reg=GROWS - 3072, elem_size=N,
                queue_num=qt,
            )
            nc.sync.dma_start(out=out_attn[3072:GROWS, :],
                              in_=gtl[:GROWS - 3072, 0, :])

    nc.compile()
    return nc


_NC_CACHE = None
TRACE = False          # set by test harness to capture an NTFF profile
LAST_RESULT = None     # BassKernelResults of the most recent kernel() call
TRACE_DIR = None


def _get_nc():
    global _NC_CACHE
    if _NC_CACHE is None:
        _NC_CACHE = build_nc()
    return _NC_CACHE


def _install_trace_hooks():
    """Register the NTFF profile hook (missing from this image's antenv)
    and keep artifacts local instead of uploading to a bucket."""
    import types
    if "antenv.axon_hooks" not in sys.modules:
        from trn_agent_boot.trn_boot import _ntff_profile_via_ctypes
        hook = _ntff_profile_via_ctypes("/opt/axon/libaxon_pjrt.so")
        mod = types.ModuleType("antenv.axon_hooks")
        mod.get_axon_ntff_profile_hook = lambda: hook
        mod.set_axon_ntff_profile_hook = lambda h: None
        sys.modules["antenv.axon_hooks"] = mod
    from concourse import bass_utils as BU
    BU.upload_artifacts = lambda tmpdir: tmpdir


def kernel(attn, value, mask, gumbel_u, output_num_tokens):
    from concourse.bass_utils import run_bass_kernel_spmd

    attn = np.ascontiguousarray(np.asarray(attn, dtype=np.float32))
    value = np.ascontiguousarray(np.asarray(value, dtype=np.float32))
    gumbel_u = np.ascontiguousarray(np.asarray(gumbel_u, dtype=np.float32))
    mask_f = np.ascontiguousarray(np.asarray(mask).astype(np.float32))
    assert int(np.asarray(output_num_tokens)) == K

    nc = _get_nc()
    in_maps = [
        {
            "attn": attn[b].reshape(NROWS, N),
            # partition-major permute so each SBUF partition loads contiguously
            "value": np.ascontiguousarray(
                value[b].reshape(96, 128, D).transpose(1, 0, 2)
            ).reshape(128, 96 * D),
            "gumbel": gumbel_u[b],
            "mask": np.ascontiguousarray(mask_f[b].reshape(8, 128).T),
        }
        for b in range(B)
    ]
    kw = {}
    if TRACE:
        import tempfile
        global TRACE_DIR
        TRACE_DIR = tempfile.mkdtemp(prefix="ats_trace_")
        kw = dict(trace=True, tmpdir=TRACE_DIR)
        _install_trace_hooks()
    res = run_bass_kernel_spmd(nc, in_maps, core_ids=list(range(B)), **kw)
    global LAST_RESULT
    LAST_RESULT = res
    results = res.results
    new_attn = np.stack([r["out_attn"].reshape(H, KP1, N) for r in results])
    new_mask = np.stack([r["out_mask"].astype(bool) for r in results])
    ids = np.stack([r["out_ids"].astype(np.int32) for r in results])
    return new_attn, new_mask, ids
